# revision 1
# baseline (speedup 1.0000x reference)
"""Trainium2 Bass kernel for nn_BaselineModel_27298812133937.

Model: two [32,512] token sequences -> shared embedding [50000,512] ->
3 stacked bi-GRU layers (H=256, Keras reset_after) -> last states,
plus a leaks MLP branch, then BN/FC/BN/sigmoid head -> [32].

Sharding: the two sequences share GRU weights, so they merge into a
batch of 64. Each of the 8 cores takes 8 merged examples (4 code + 4
comment of the same original examples), runs the full network for its
shard with no cross-core communication, and computes the head for its
4 original examples. Host concatenates the 8x[4] outputs.

On-core layout: everything "transposed" (feature dim on partitions,
batch on the free dim). The recurrent matmul keeps Wh (bf16) as the
stationary operand and streams the state, producing rec^T in PSUM so
all gate math runs on [128, j, B] tiles.
"""

import os
import sys

import numpy as np

for _p in ("/opt/trn_rl_repo",):
    if os.path.isdir(_p) and _p not in sys.path:
        sys.path.insert(0, _p)

import concourse.bass as bass
import concourse.tile as tile
from concourse import bacc, mybir
from concourse.masks import make_identity

import ml_dtypes

FP32 = mybir.dt.float32
BF16 = mybir.dt.bfloat16
I32 = mybir.dt.int32
AF = mybir.ActivationFunctionType
OP = mybir.AluOpType
NP_BF16 = ml_dtypes.bfloat16

V, E, H, NLAY = 50000, 512, 256, 3
EPS = 1e-3
P = 128
JX = E // P        # 4  x-feature tiles
JG = 3 * H // P    # 6  gate tiles
JH = H // P        # 2  hidden tiles
KH = H // P        # 2  Wh contraction tiles
BC = 8             # merged examples per core
BCH = 4            # head (original) examples per core
NCORES = 8
U = 8              # scan unroll == xp time-block


def build_nc(T=512, n_layers=NLAY, use_for_i=True, staggered=True, debug=False):
    assert T % P == 0 and T % U == 0
    TB = T // U
    NCH = BC * (T // P)

    nc = bacc.Bacc("TRN2", target_bir_lowering=False, debug=debug)

    def din(name, shape, dt):
        return nc.declare_dram_parameter(name, list(shape), dt, False)

    emb = din("emb", [V, E], BF16)
    idxw = din("idxw", [P, NCH], I32)
    wx = din("wx", [n_layers, 2, JX, JG, P, P], BF16)
    wh = din("wh", [n_layers, 2, KH, JG, P, P], BF16)
    pbias = din("pbias", [P, n_layers, 2, JG], FP32)
    b1h = din("b1h", [P, n_layers, 2, JH], FP32)
    w1 = din("w1", [10, 2, P, P], BF16)
    b1p = din("b1p", [P, 2], FP32)
    wc = din("wc", [P, 2], BF16)
    bc_b = din("bc", [1, 1], FP32)
    lw0 = din("lw0", [P, 2, P], BF16)
    lw1 = din("lw1", [20, 2, P], BF16)
    lb = din("lb", [P, 2], FP32)
    leakst = din("leakst", [148, BCH], BF16)

    out = nc.declare_dram_parameter("out", [1, BCH], FP32, True)

    # internal DRAM
    x_bufs = [nc.dram_tensor(f"x{i}", [P, JX, T, BC], BF16) for i in range(2)]
    # xp buffers; bw (d=1) is stored already reversed in time so the scan
    # indexes both dirs identically.
    xpz = [nc.dram_tensor(f"xpz{d}", [P, TB, 4, BC, U], FP32) for d in range(2)]
    xph = [nc.dram_tensor(f"xph{d}", [P, TB, 2, BC, U], FP32) for d in range(2)]

    with tile.TileContext(nc) as tc, tc.tile_pool(name="const", bufs=1) as cpool:
        # ---- constants in SBUF
        ident = cpool.tile([P, P], BF16)
        make_identity(nc, ident[:])
        idx_sb = cpool.tile([P, NCH], I32)
        nc.sync.dma_start(idx_sb[:], idxw[:])
        pb_sb = cpool.tile([P, n_layers, 2, JG], FP32)
        nc.sync.dma_start(pb_sb[:], pbias[:])
        b1h_sb = cpool.tile([P, n_layers, 2, JH], FP32)
        nc.sync.dma_start(b1h_sb[:], b1h[:])
        fin_hold = [cpool.tile([P, JH, 1, BC], BF16, name=f"fin{i}") for i in range(2)]

        # ---- phase 1: embedding gather -> x0 (layer-0 input, transposed)
        x0 = x_bufs[0]
        with (
            tc.tile_pool(name="erow", bufs=3) as epool,
            tc.tile_pool(name="estage", bufs=2) as espool,
            tc.tile_pool(name="epsum", bufs=4, space="PSUM") as eppool,
        ):
            for tc_i in range(T // P):
                stages = [
                    espool.tile([P, P, BC], BF16, name=f"estg{j}", tag=f"st{j}")
                    for j in range(JX)
                ]
                for bi in range(BC):
                    ch = bi * (T // P) + tc_i
                    g = epool.tile([P, E], BF16)
                    nc.gpsimd.indirect_dma_start(
                        out=g[:],
                        out_offset=None,
                        in_=emb[:],
                        in_offset=bass.IndirectOffsetOnAxis(
                            ap=idx_sb[:, ch : ch + 1], axis=0
                        ),
                    )
                    for j in range(JX):
                        pst = eppool.tile([P, P], BF16)
                        nc.tensor.transpose(pst[:], g[:, j * P : (j + 1) * P], ident[:])
                        if (bi + j) % 2 == 0:
                            nc.vector.tensor_copy(stages[j][:, :, bi], pst[:])
                        else:
                            nc.scalar.copy(stages[j][:, :, bi], pst[:])
                for j in range(JX):
                    nc.sync.dma_start(
                        x0[:, j, tc_i * P : (tc_i + 1) * P, :], stages[j][:]
                    )

        # ---- per-layer: proj (both dirs) then scan (both dirs)
        for l in range(n_layers):
            x_cur = x_bufs[l % 2]
            x_next = x_bufs[(l + 1) % 2]
            is_last = l == n_layers - 1

            # -- input projection: xp^T = Wx^T @ x^T (+bias), to DRAM
            with (
                tc.tile_pool(name="wts", bufs=1) as wpool,
                tc.tile_pool(name="pstage", bufs=3) as pspool,
                tc.tile_pool(name="ppsum", bufs=2, space="PSUM") as pppool,
                tc.tile_pool(name="xchunk", bufs=2) as xcpool,
            ):
                wx_sb = wpool.tile([P, 2, JX, JG, P], BF16)
                nc.sync.dma_start(wx_sb[:], wx[l].rearrange("d kt mt p q -> p d kt mt q"))

                NCK = T // 64  # chunks of 512 cols (64 t x 8 b)
                for d in range(2):
                    for c in range(NCK):
                        xch = xcpool.tile([P, JX, 64, BC], BF16, tag="xch")
                        nc.sync.dma_start(xch[:], x_cur[:, :, c * 64 : (c + 1) * 64, :])
                        for mt in range(JG):
                            ps = pppool.tile([P, 512], FP32)
                            for kt in range(JX):
                                nc.tensor.matmul(
                                    ps[:],
                                    wx_sb[:, d, kt, mt, :],
                                    xch[:, kt, :, :],
                                    start=(kt == 0),
                                    stop=(kt == JX - 1),
                                )
                            # staging layout [P, tb, b, ti]; psum iter order is
                            # (t, b) = (tb, ti, b) -> permuted write AP
                            stg = pspool.tile([P, 8, BC, U], FP32, tag="stg")
                            if d == 0:
                                stg_w = stg[:].rearrange("p tb b ti -> p tb ti b")
                                tb_lo = c * 8
                                dst_tb = slice(tb_lo, tb_lo + 8)
                            else:
                                # bw: store reversed in time (block and
                                # within-block order both reversed) by writing
                                # the staging tile through a reversed AP
                                stg_w = stg[:, ::-1, :, ::-1].rearrange(
                                    "p tb b ti -> p tb ti b"
                                )
                                tb_hi = TB - c * 8
                                dst_tb = slice(tb_hi - 8, tb_hi)
                            if mt < 4:
                                nc.vector.tensor_scalar_add(
                                    stg_w, ps[:], pb_sb[:, l, d, mt : mt + 1]
                                )
                                dst = xpz[d][:, dst_tb, mt, :, :]
                            else:
                                nc.scalar.activation(
                                    stg_w,
                                    ps[:],
                                    AF.Identity,
                                    bias=pb_sb[:, l, d, mt : mt + 1],
                                )
                                dst = xph[d][:, dst_tb, mt - 4, :, :]
                            nc.sync.dma_start(dst, stg[:])

            # -- scan
            with (
                tc.tile_pool(name="state", bufs=4) as stpool,
                tc.tile_pool(name="gates", bufs=3) as gpool,
                tc.tile_pool(name="xpchunk", bufs=3) as xppool,
                tc.tile_pool(name="spsum", bufs=4, space="PSUM") as sppool,
                tc.tile_pool(name="wts2", bufs=1) as wpool2,
            ):
                wh_sb = wpool2.tile([P, 2, KH, JG, P], BF16)
                nc.sync.dma_start(wh_sb[:], wh[l].rearrange("d kt mt p q -> p d kt mt q"))

                # fixed double-buffered state tiles: u even reads A writes B
                stAB = []
                for d in range(2):
                    stA = stpool.tile([P, JH, 1, BC], BF16, name=f"stA{d}")
                    stB = stpool.tile([P, JH, 1, BC], BF16, name=f"stB{d}")
                    nc.vector.memset(stA[:], 0.0)
                    stAB.append((stA, stB))

                def scan_block(ib, dyn):
                    xz_ch = []
                    xh_ch = []
                    for d in range(2):
                        bsl = bass.ds(ib, 1) if dyn else slice(ib, ib + 1)
                        xz = xppool.tile([P, 1, 4, BC, U], FP32, tag=f"xz{d}")
                        nc.sync.dma_start(xz[:], xpz[d][:, bsl, :, :, :])
                        xh_ = xppool.tile([P, 1, 2, BC, U], FP32, tag=f"xh{d}")
                        nc.sync.dma_start(xh_[:], xph[d][:, bsl, :, :, :])
                        xz_ch.append(xz)
                        xh_ch.append(xh_)
                    for u in range(U):
                        for d in range(2):
                            stp = stAB[d][u % 2]
                            stn = stAB[d][(u + 1) % 2]
                            pt = sppool.tile([P, JG, BC], FP32, tag=f"ps{d}")
                            for mt in range(JG):
                                for kt in range(KH):
                                    nc.tensor.matmul(
                                        pt[:, mt, :],
                                        wh_sb[:, d, kt, mt, :],
                                        stp[:, kt, 0, :],
                                        start=(kt == 0),
                                        stop=(kt == KH - 1),
                                    )
                            pre = gpool.tile([P, 4, BC], FP32, tag=f"pre{d}")
                            nc.vector.scalar_tensor_tensor(
                                pre[:],
                                pt[:, 0:4, :],
                                0.0,
                                xz_ch[d][:, 0, :, :, u],
                                OP.add,
                                OP.add,
                            )
                            zr = gpool.tile([P, 4, BC], FP32, tag=f"zr{d}")
                            nc.scalar.activation(zr[:], pre[:], AF.Sigmoid)
                            hm = gpool.tile([P, 2, BC], FP32, tag=f"hm{d}")
                            for j in range(JH):
                                nc.vector.scalar_tensor_tensor(
                                    hm[:, j, :],
                                    pt[:, 4 + j, :],
                                    b1h_sb[:, l, d, j : j + 1],
                                    zr[:, 2 + j, :],
                                    OP.add,
                                    OP.mult,
                                )
                            av = gpool.tile([P, 2, BC], FP32, tag=f"av{d}")
                            nc.vector.tensor_tensor(
                                av[:], hm[:], xh_ch[d][:, 0, :, :, u], OP.add
                            )
                            hh = gpool.tile([P, 2, BC], FP32, tag=f"hh{d}")
                            nc.scalar.activation(hh[:], av[:], AF.Tanh)
                            dd = gpool.tile([P, 2, BC], FP32, tag=f"dd{d}")
                            nc.vector.tensor_tensor(
                                dd[:], stp[:, :, 0, :], hh[:], OP.subtract
                            )
                            ee = gpool.tile([P, 2, BC], FP32, tag=f"ee{d}")
                            nc.vector.tensor_tensor(
                                ee[:], zr[:, 0:2, :], dd[:], OP.mult
                            )
                            nc.vector.tensor_tensor(
                                stn[:, :, 0, :], hh[:], ee[:], OP.add
                            )
                            if not is_last:
                                # fw wrote forward time, bw writes reversed
                                if dyn:
                                    if d == 0:
                                        t_ap = bass.ds(ib * U + u, 1)
                                    else:
                                        t_ap = bass.ds(
                                            ib * (-U) + (T - 1 - u), 1
                                        )
                                else:
                                    t0 = ib * U + u if d == 0 else T - 1 - ib * U - u
                                    t_ap = slice(t0, t0 + 1)
                                nc.sync.dma_start(
                                    x_next[:, 2 * d : 2 * d + 2, t_ap, :], stn[:]
                                )

                if use_for_i:
                    with tc.For_i(0, TB, 1, staggered_reset=staggered) as ib:
                        scan_block(ib, True)
                else:
                    for ib in range(TB):
                        scan_block(ib, False)

                if is_last:
                    for d in range(2):
                        nc.vector.tensor_copy(fin_hold[d][:], stAB[d][0][:])

        # ---- head: leaks branch + folded BN/FC/BN/sigmoid
        with (
            tc.tile_pool(name="head", bufs=1) as hpool,
            tc.tile_pool(name="hpsum", bufs=2, space="PSUM") as hppool,
        ):
            lkw0 = hpool.tile([P, 2, P], BF16)
            nc.sync.dma_start(lkw0[:], lw0[:])
            lkw1 = hpool.tile([20, 2, P], BF16)
            nc.sync.dma_start(lkw1[:], lw1[:])
            lkb = hpool.tile([P, 2], FP32)
            nc.sync.dma_start(lkb[:], lb[:])
            lkx0 = hpool.tile([P, BCH], BF16)
            nc.sync.dma_start(lkx0[:], leakst[0:P, :])
            lkx1 = hpool.tile([20, BCH], BF16)
            nc.sync.dma_start(lkx1[:], leakst[P:148, :])

            lks = hpool.tile([P, 2, BCH], BF16)
            for mt in range(2):
                lp = hppool.tile([P, BCH], FP32, tag="lp")
                nc.tensor.matmul(lp[:], lkw0[:, mt, :], lkx0[:], start=True, stop=False)
                nc.tensor.matmul(lp[:], lkw1[:, mt, :], lkx1[:], start=False, stop=True)
                nc.scalar.activation(
                    lks[:, mt, :], lp[:], AF.Relu, bias=lkb[:, mt : mt + 1]
                )

            w1_sb = hpool.tile([P, 10, 2, P], BF16)
            nc.sync.dma_start(w1_sb[:], w1[:].rearrange("kt mt p q -> p kt mt q"))
            b1_sb = hpool.tile([P, 2], FP32)
            nc.sync.dma_start(b1_sb[:], b1p[:])
            wc_sb = hpool.tile([P, 2], BF16)
            nc.sync.dma_start(wc_sb[:], wc[:])
            bc_sb = hpool.tile([1, 1], FP32)
            nc.sync.dma_start(bc_sb[:], bc_b[:])

            sf, sb_ = fin_hold
            rhs_tiles = []
            for half in range(2):  # code (cols 0:4), comment (cols 4:8)
                c0 = half * BCH
                for dstate in (sf, sb_):
                    for j in range(JH):
                        rhs_tiles.append(dstate[:, j, 0, c0 : c0 + BCH])
            rhs_tiles.append(lks[:, 0, :])
            rhs_tiles.append(lks[:, 1, :])

            yt = hpool.tile([P, 2, BCH], BF16)
            for mt in range(2):
                hp = hppool.tile([P, BCH], FP32, tag="hp")
                for kt in range(10):
                    nc.tensor.matmul(
                        hp[:],
                        w1_sb[:, kt, mt, :],
                        rhs_tiles[kt],
                        start=(kt == 0),
                        stop=(kt == 9),
                    )
                nc.scalar.activation(
                    yt[:, mt, :], hp[:], AF.Relu, bias=b1_sb[:, mt : mt + 1]
                )

            op_ = hppool.tile([1, BCH], FP32, tag="op")
            for kt in range(2):
                nc.tensor.matmul(
                    op_[:],
                    wc_sb[:, kt : kt + 1],
                    yt[:, kt, :],
                    start=(kt == 0),
                    stop=(kt == 1),
                )
            res = hpool.tile([1, BCH], FP32)
            nc.scalar.activation(res[:], op_[:], AF.Sigmoid, bias=bc_sb[0:1, 0:1])
            nc.sync.dma_start(out[:], res[:])

    nc.compile()
    return nc


def prep_inputs(inputs, T=512, n_layers=NLAY):
    """Host-side: shard + pre-layout all tensors. Returns in_maps list."""
    ci = np.asarray(inputs["comment_indices"]).astype(np.int32)
    co = np.asarray(inputs["code_indices"]).astype(np.int32)
    emb_bf = np.ascontiguousarray(
        np.asarray(inputs["embed_table"], np.float32)
    ).astype(NP_BF16)
    gwx = np.asarray(inputs["gru_Wx"], np.float32)
    gwh = np.asarray(inputs["gru_Wh"], np.float32)
    gb = np.asarray(inputs["gru_b"], np.float32)

    wx_t = np.ascontiguousarray(
        gwx[:n_layers].reshape(n_layers, 2, JX, P, JG, P).transpose(0, 1, 2, 4, 3, 5)
    ).astype(NP_BF16)
    wh_t = np.ascontiguousarray(
        gwh[:n_layers].reshape(n_layers, 2, KH, P, JG, P).transpose(0, 1, 2, 4, 3, 5)
    ).astype(NP_BF16)

    pb = gb[:n_layers, :, 0, :].copy()  # [nl, 2, 768]
    pb[:, :, : 2 * H] += gb[:n_layers, :, 1, : 2 * H]
    pbias_h = np.ascontiguousarray(
        pb.reshape(n_layers, 2, JG, P).transpose(3, 0, 1, 2)
    ).astype(np.float32)
    b1h_h = np.ascontiguousarray(
        gb[:n_layers, :, 1, 2 * H :].reshape(n_layers, 2, JH, P).transpose(3, 0, 1, 2)
    ).astype(np.float32)

    s1 = np.asarray(inputs["bn1_gamma"], np.float32) / np.sqrt(
        np.asarray(inputs["bn1_var"], np.float32) + EPS
    )
    t1 = (
        np.asarray(inputs["bn1_beta"], np.float32)
        - np.asarray(inputs["bn1_mean"], np.float32) * s1
    )
    fc1 = np.asarray(inputs["fc1_W"], np.float32)
    w1p = fc1 * s1[:, None]
    b1v = t1 @ fc1 + np.asarray(inputs["fc1_b"], np.float32)
    s2 = np.asarray(inputs["bn2_gamma"], np.float32) / np.sqrt(
        np.asarray(inputs["bn2_var"], np.float32) + EPS
    )
    t2 = (
        np.asarray(inputs["bn2_beta"], np.float32)
        - np.asarray(inputs["bn2_mean"], np.float32) * s2
    )
    clsw = np.asarray(inputs["cls_W"], np.float32)
    wcp = clsw * s2[:, None]
    bcp = (t2 @ clsw + np.asarray(inputs["cls_b"], np.float32)).reshape(1, 1)

    w1_t = np.ascontiguousarray(w1p.reshape(10, P, 2, P).transpose(0, 2, 1, 3)).astype(
        NP_BF16
    )
    b1p_h = np.ascontiguousarray(b1v.reshape(2, P).T).astype(np.float32)
    wc_h = np.ascontiguousarray(wcp.reshape(2, P).T).astype(NP_BF16)

    lw = np.asarray(inputs["leaks_W"], np.float32)
    lw0_h = np.ascontiguousarray(lw[:P].reshape(P, 2, P)).astype(NP_BF16)
    lw1_h = np.ascontiguousarray(lw[P:].reshape(20, 2, P)).astype(NP_BF16)
    lb_h = np.ascontiguousarray(
        np.asarray(inputs["leaks_b"], np.float32).reshape(2, P).T
    ).astype(np.float32)
    leaks = np.asarray(inputs["leaks_indices"], np.float32)

    shared = dict(
        emb=emb_bf, wx=wx_t, wh=wh_t, pbias=pbias_h, b1h=b1h_h,
        w1=w1_t, b1p=b1p_h, wc=wc_h, bc=bcp.astype(np.float32),
        lw0=lw0_h, lw1=lw1_h, lb=lb_h,
    )
    in_maps = []
    for c in range(NCORES):
        exs = slice(BCH * c, BCH * c + BCH)
        merged = np.concatenate([co[exs, :T], ci[exs, :T]], 0)  # [8, T]
        idxw_h = np.ascontiguousarray(
            merged.reshape(BC, T // P, P).transpose(2, 0, 1).reshape(P, -1)
        ).astype(np.int32)
        lkt = np.ascontiguousarray(leaks[exs].T).astype(NP_BF16)
        m = dict(shared)
        m["idxw"] = idxw_h
        m["leakst"] = lkt
        in_maps.append(m)
    return in_maps


def kernel(**inputs) -> np.ndarray:
    from concourse.bass_utils import run_bass_kernel_spmd

    nc = build_nc(T=512)
    in_maps = prep_inputs(inputs, T=512)
    res = run_bass_kernel_spmd(nc, in_maps, list(range(NCORES)))
    outs = [np.asarray(res.results[c]["out"]).reshape(-1) for c in range(NCORES)]
    return np.concatenate(outs).astype(np.float32)


if __name__ == "__main__":
    sys.path.insert(0, "/root/problem")
    import reference

    inp = {k: np.asarray(v) for k, v in reference.setup_inputs().items()}
    got = kernel(**inp)
    print("kernel out:", got[:8])



# revision 8
# speedup vs baseline: 1.0416x; 1.0416x over previous
"""Trainium2 Bass kernel for nn_BaselineModel_27298812133937.

Model: two [32,512] token sequences -> shared embedding [50000,512] ->
3 stacked bi-GRU layers (H=256, Keras reset_after) -> last states,
plus a leaks MLP branch, then BN/FC/BN/sigmoid head -> [32].

Sharding: the two sequences share GRU weights, so they merge into a
batch of 64. Each of the 8 cores takes 8 merged examples (4 code + 4
comment of the same original examples), runs the full network for its
shard with no cross-core communication, and computes the head for its
4 original examples. Host concatenates the 8x[4] outputs.

v3 scan dataflow (trace-driven): tiny matmuls issue every ~27ns on the
PE (the 167ns slice is pipeline latency), so PE work is cheap while the
DVE and the serial gate chain dominate. Per step the precomputed input
projection xz is accumulated into PSUM via an identity matmul, the
recurrent h-gate bias via a K=1 bias-row matmul, so the gate math is
down to sigmoid/tanh on the Act engine plus 5 tensor_tensor ops that
run on DVE for the fw direction and GpSimd for the bw direction. The
new state is written directly into a bf16 staging tile that both the
next step's matmul and the x_next DMA read.
"""

import os
import sys

import numpy as np

for _p in ("/opt/trn_rl_repo",):
    if os.path.isdir(_p) and _p not in sys.path:
        sys.path.insert(0, _p)

import concourse.bass as bass
import concourse.tile as tile
from concourse import bacc, mybir
from concourse.masks import make_identity

import ml_dtypes

FP32 = mybir.dt.float32
BF16 = mybir.dt.bfloat16
I32 = mybir.dt.int32
AF = mybir.ActivationFunctionType
OP = mybir.AluOpType
NP_BF16 = ml_dtypes.bfloat16

V, E, H, NLAY = 50000, 512, 256, 3
EPS = 1e-3
P = 128
JX = E // P        # 4  x-feature tiles
JG = 3 * H // P    # 6  gate tiles
JH = H // P        # 2  hidden tiles
KH = H // P        # 2  Wh contraction tiles
BC = 8             # merged examples per core
BCH = 4            # head (original) examples per core
NCORES = 8
U = 8              # scan unroll == xp time-block


def build_nc(T=512, n_layers=NLAY, use_for_i=True, staggered=True, debug=False):
    assert T % P == 0 and T % (2 * U) == 0
    TB = T // U
    TB2 = T // (2 * U)
    NCH = BC * (T // P)

    nc = bacc.Bacc("TRN2", target_bir_lowering=False, debug=debug)

    def din(name, shape, dt):
        return nc.declare_dram_parameter(name, list(shape), dt, False)

    emb = din("emb", [V, E], BF16)
    idxw = din("idxw", [P, NCH], I32)
    wx = din("wx", [n_layers, 2, JX, JG, P, P], BF16)
    wh = din("wh", [n_layers, 2, KH, JG, P, P], BF16)
    pbias = din("pbias", [P, n_layers, 2, JG], FP32)
    b1row = din("b1row", [1, n_layers, 2, JH, P], BF16)
    w1 = din("w1", [10, 2, P, P], BF16)
    b1p = din("b1p", [P, 2], FP32)
    wc = din("wc", [P, 2], BF16)
    bc_b = din("bc", [1, 1], FP32)
    lw0 = din("lw0", [P, 2, P], BF16)
    lw1 = din("lw1", [20, 2, P], BF16)
    lb = din("lb", [P, 2], FP32)
    leakst = din("leakst", [148, BCH], BF16)

    out = nc.declare_dram_parameter("out", [1, BCH], FP32, True)

    # internal DRAM
    x_bufs = [nc.dram_tensor(f"x{i}", [P, JX, T, BC], BF16) for i in range(2)]
    # xp: merged projection (zr gates 0:4 with full bias; xh 4:6 with input
    # bias). bw (d=1) stored time-reversed so the scan indexes both dirs
    # identically.
    xp = [nc.dram_tensor(f"xp{d}", [P, TB, JG, BC, U], BF16) for d in range(2)]

    with tile.TileContext(nc) as tc, tc.tile_pool(name="const", bufs=1) as cpool:
        # ---- constants in SBUF
        ident = cpool.tile([P, P], BF16)
        make_identity(nc, ident[:])
        idx_sb = cpool.tile([P, NCH], I32)
        nc.sync.dma_start(idx_sb[:], idxw[:])
        pb_sb = cpool.tile([P, n_layers, 2, JG], FP32)
        nc.sync.dma_start(pb_sb[:], pbias[:])
        b1r_sb = cpool.tile([1, n_layers, 2, JH, P], BF16)
        nc.sync.dma_start(b1r_sb[:], b1row[:])
        ones_sb = cpool.tile([1, BC], BF16)
        nc.vector.memset(ones_sb[:], 1.0)
        fin_hold = [cpool.tile([P, JH, BC], BF16, name=f"fin{i}") for i in range(2)]

        # ---- phase 1: embedding gather -> x0 (layer-0 input, transposed)
        x0 = x_bufs[0]
        with (
            tc.tile_pool(name="erow", bufs=3) as epool,
            tc.tile_pool(name="estage", bufs=2) as espool,
            tc.tile_pool(name="epsum", bufs=4, space="PSUM") as eppool,
        ):
            for tc_i in range(T // P):
                stages = [
                    espool.tile([P, P, BC], BF16, name=f"estg{j}", tag=f"st{j}")
                    for j in range(JX)
                ]
                for bi in range(BC):
                    ch = bi * (T // P) + tc_i
                    g = epool.tile([P, E], BF16)
                    nc.gpsimd.indirect_dma_start(
                        out=g[:],
                        out_offset=None,
                        in_=emb[:],
                        in_offset=bass.IndirectOffsetOnAxis(
                            ap=idx_sb[:, ch : ch + 1], axis=0
                        ),
                    )
                    for j in range(JX):
                        pst = eppool.tile([P, P], BF16)
                        nc.tensor.transpose(pst[:], g[:, j * P : (j + 1) * P], ident[:])
                        if (bi + j) % 2 == 0:
                            nc.vector.tensor_copy(stages[j][:, :, bi], pst[:])
                        else:
                            nc.scalar.copy(stages[j][:, :, bi], pst[:])
                for j in range(JX):
                    nc.sync.dma_start(
                        x0[:, j, tc_i * P : (tc_i + 1) * P, :], stages[j][:]
                    )

        # ---- per-layer: proj (both dirs) then scan (both dirs)
        for l in range(n_layers):
            x_cur = x_bufs[l % 2]
            x_next = x_bufs[(l + 1) % 2]
            is_last = l == n_layers - 1

            # -- input projection: xp^T = Wx^T @ x^T (+bias), to DRAM (bf16)
            with (
                tc.tile_pool(name="wts", bufs=1) as wpool,
                tc.tile_pool(name="pstage", bufs=3) as pspool,
                tc.tile_pool(name="ppsum", bufs=2, space="PSUM") as pppool,
                tc.tile_pool(name="xchunk", bufs=2) as xcpool,
            ):
                wx_sb = wpool.tile([P, 2, JX, JG, P], BF16)
                nc.sync.dma_start(wx_sb[:], wx[l].rearrange("d kt mt p q -> p d kt mt q"))

                NCK = T // 64  # chunks of 512 cols (64 t x 8 b)
                for d in range(2):
                    for c in range(NCK):
                        xch = xcpool.tile([P, JX, 64, BC], BF16, tag="xch")
                        nc.sync.dma_start(xch[:], x_cur[:, :, c * 64 : (c + 1) * 64, :])
                        for mt in range(JG):
                            ps = pppool.tile([P, 512], FP32)
                            for kt in range(JX):
                                nc.tensor.matmul(
                                    ps[:],
                                    wx_sb[:, d, kt, mt, :],
                                    xch[:, kt, :, :],
                                    start=(kt == 0),
                                    stop=(kt == JX - 1),
                                )
                            # staging layout [P, tb, b, ti]; psum iter order is
                            # (t, b) = (tb, ti, b) -> permuted write AP
                            stg = pspool.tile([P, 8, BC, U], BF16, tag="stg")
                            if d == 0:
                                stg_w = stg[:].rearrange("p tb b ti -> p tb ti b")
                                tb_lo = c * 8
                                dst_tb = slice(tb_lo, tb_lo + 8)
                            else:
                                # bw: store reversed in time (block and
                                # within-block order both reversed) by writing
                                # the staging tile through a reversed AP
                                stg_w = stg[:, ::-1, :, ::-1].rearrange(
                                    "p tb b ti -> p tb ti b"
                                )
                                tb_hi = TB - c * 8
                                dst_tb = slice(tb_hi - 8, tb_hi)
                            if mt % 2 == 0:
                                nc.vector.tensor_scalar_add(
                                    stg_w, ps[:], pb_sb[:, l, d, mt : mt + 1]
                                )
                            else:
                                nc.scalar.activation(
                                    stg_w,
                                    ps[:],
                                    AF.Identity,
                                    bias=pb_sb[:, l, d, mt : mt + 1],
                                )
                            nc.sync.dma_start(xp[d][:, dst_tb, mt, :, :], stg[:])

            # -- scan
            with (
                tc.tile_pool(name="state", bufs=4) as stpool,
                tc.tile_pool(name="gates", bufs=3) as gpool,
                tc.tile_pool(name="xpchunk", bufs=3) as xppool,
                tc.tile_pool(name="spsum", bufs=4, space="PSUM") as sppool,
                tc.tile_pool(name="wts2", bufs=1) as wpool2,
            ):
                wh_sb = wpool2.tile([P, 2, KH, JG, P], BF16)
                nc.sync.dma_start(wh_sb[:], wh[l].rearrange("d kt mt p q -> p d kt mt q"))

                # fixed staging tiles: state slots double as x_next staging.
                # body handles 2 sub-blocks (A then B); A's u=0 reads B's
                # last slot from the previous iteration.
                stg = []  # [d][sub] -> tile [P, JH, U, BC] bf16
                for d in range(2):
                    sA = stpool.tile([P, JH, U, BC], BF16, name=f"stgA{d}")
                    sB = stpool.tile([P, JH, U, BC], BF16, name=f"stgB{d}")
                    nc.vector.memset(sB[:], 0.0)
                    stg.append((sA, sB))

                def scan_sub(ib, sub, dyn):
                    # chunk loads (one per dir): [P, JG, BC, U] bf16
                    chunks = []
                    for d in range(2):
                        if dyn:
                            tbs = bass.ds(ib * 2 + sub, 1)
                        else:
                            tbs = slice(ib * 2 + sub, ib * 2 + sub + 1)
                        xc = xppool.tile([P, 1, JG, BC, U], BF16, tag=f"xc{d}{sub}")
                        nc.sync.dma_start(xc[:], xp[d][:, tbs, :, :, :])
                        chunks.append(xc)
                    cur = [stg[d][sub] for d in range(2)]
                    prev = [stg[d][1 - sub] for d in range(2)]
                    for u in range(U):
                        for d in range(2):
                            # fw fills slots ascending (slot u == time base+u);
                            # bw fills descending (slot U-1-u) so the tile ends
                            # up in ascending-time order for a direct DMA.
                            if d == 0:
                                slot_w = u
                                vprev = (
                                    cur[d][:, :, u - 1, :]
                                    if u > 0
                                    else prev[d][:, :, U - 1, :]
                                )
                            else:
                                slot_w = U - 1 - u
                                vprev = (
                                    cur[d][:, :, U - u, :]
                                    if u > 0
                                    else prev[d][:, :, 0, :]
                                )
                            xz = chunks[d][:, 0, 0:4, :, u]
                            xh = chunks[d][:, 0, 4:6, :, u]
                            pt = sppool.tile([P, JG, BC], FP32, tag=f"pt{d}")
                            # zr psum: xz via identity, then Wh accumulation
                            for mt in range(4):
                                nc.tensor.matmul(
                                    pt[:, mt, :], ident[:], xz[:, mt, :],
                                    start=True, stop=False,
                                )
                                for kt in range(KH):
                                    nc.tensor.matmul(
                                        pt[:, mt, :],
                                        wh_sb[:, d, kt, mt, :],
                                        vprev[:, kt, :],
                                        start=False, stop=(kt == KH - 1),
                                    )
                            # h psum: recurrent bias row, then Wh accumulation
                            for jh in range(JH):
                                nc.tensor.matmul(
                                    pt[:, 4 + jh, :],
                                    b1r_sb[0:1, l, d, jh, :],
                                    ones_sb[0:1, :],
                                    start=True, stop=False,
                                )
                                for kt in range(KH):
                                    nc.tensor.matmul(
                                        pt[:, 4 + jh, :],
                                        wh_sb[:, d, kt, 4 + jh, :],
                                        vprev[:, kt, :],
                                        start=False, stop=(kt == KH - 1),
                                    )
                            # gpsimd cannot touch PSUM: hm always on DVE,
                            # the rest of the bw chain on gpsimd.
                            eng = nc.vector if d == 0 else nc.gpsimd
                            zr = gpool.tile([P, 4, BC], FP32, tag=f"zr{d}")
                            nc.scalar.activation(zr[:], pt[:, 0:4, :], AF.Sigmoid)
                            hm = gpool.tile([P, 2, BC], FP32, tag=f"hm{d}")
                            nc.vector.tensor_tensor(
                                hm[:], pt[:, 4:6, :], zr[:, 2:4, :], OP.mult
                            )
                            av = gpool.tile([P, 2, BC], FP32, tag=f"av{d}")
                            eng.tensor_tensor(av[:], hm[:], xh, OP.add)
                            hh = gpool.tile([P, 2, BC], FP32, tag=f"hh{d}")
                            nc.scalar.activation(hh[:], av[:], AF.Tanh)
                            dd = gpool.tile([P, 2, BC], FP32, tag=f"dd{d}")
                            eng.tensor_tensor(dd[:], vprev, hh[:], OP.subtract)
                            ee = gpool.tile([P, 2, BC], FP32, tag=f"ee{d}")
                            eng.tensor_tensor(ee[:], zr[:, 0:2, :], dd[:], OP.mult)
                            eng.tensor_tensor(
                                cur[d][:, :, slot_w, :], hh[:], ee[:], OP.add
                            )
                    if not is_last:
                        for d in range(2):
                            if d == 0:
                                if dyn:
                                    t_ap = bass.ds(ib * (2 * U) + sub * U, U)
                                else:
                                    t0 = ib * 2 * U + sub * U
                                    t_ap = slice(t0, t0 + U)
                            else:
                                if dyn:
                                    t_ap = bass.ds(
                                        ib * (-2 * U) + (T - U - sub * U), U
                                    )
                                else:
                                    t0 = T - U - sub * U - ib * 2 * U
                                    t_ap = slice(t0, t0 + U)
                            nc.sync.dma_start(
                                x_next[:, 2 * d : 2 * d + 2, t_ap, :], cur[d][:]
                            )

                if use_for_i:
                    with tc.For_i(0, TB2, 1, staggered_reset=staggered) as ib:
                        scan_sub(ib, 0, True)
                        scan_sub(ib, 1, True)
                else:
                    for ib in range(TB2):
                        scan_sub(ib, 0, False)
                        scan_sub(ib, 1, False)

                if is_last:
                    # final state: fw in last slot, bw in slot 0 (descending fill)
                    nc.vector.tensor_copy(fin_hold[0][:], stg[0][1][:, :, U - 1, :])
                    nc.vector.tensor_copy(fin_hold[1][:], stg[1][1][:, :, 0, :])

        # ---- head: leaks branch + folded BN/FC/BN/sigmoid
        with (
            tc.tile_pool(name="head", bufs=1) as hpool,
            tc.tile_pool(name="hpsum", bufs=2, space="PSUM") as hppool,
        ):
            lkw0 = hpool.tile([P, 2, P], BF16)
            nc.sync.dma_start(lkw0[:], lw0[:])
            lkw1 = hpool.tile([20, 2, P], BF16)
            nc.sync.dma_start(lkw1[:], lw1[:])
            lkb = hpool.tile([P, 2], FP32)
            nc.sync.dma_start(lkb[:], lb[:])
            lkx0 = hpool.tile([P, BCH], BF16)
            nc.sync.dma_start(lkx0[:], leakst[0:P, :])
            lkx1 = hpool.tile([20, BCH], BF16)
            nc.sync.dma_start(lkx1[:], leakst[P:148, :])

            lks = hpool.tile([P, 2, BCH], BF16)
            for mt in range(2):
                lp = hppool.tile([P, BCH], FP32, tag="lp")
                nc.tensor.matmul(lp[:], lkw0[:, mt, :], lkx0[:], start=True, stop=False)
                nc.tensor.matmul(lp[:], lkw1[:, mt, :], lkx1[:], start=False, stop=True)
                nc.scalar.activation(
                    lks[:, mt, :], lp[:], AF.Relu, bias=lkb[:, mt : mt + 1]
                )

            w1_sb = hpool.tile([P, 10, 2, P], BF16)
            nc.sync.dma_start(w1_sb[:], w1[:].rearrange("kt mt p q -> p kt mt q"))
            b1_sb = hpool.tile([P, 2], FP32)
            nc.sync.dma_start(b1_sb[:], b1p[:])
            wc_sb = hpool.tile([P, 2], BF16)
            nc.sync.dma_start(wc_sb[:], wc[:])
            bc_sb = hpool.tile([1, 1], FP32)
            nc.sync.dma_start(bc_sb[:], bc_b[:])

            sf, sb_ = fin_hold
            rhs_tiles = []
            for half in range(2):  # code (cols 0:4), comment (cols 4:8)
                c0 = half * BCH
                for dstate in (sf, sb_):
                    for j in range(JH):
                        rhs_tiles.append(dstate[:, j, c0 : c0 + BCH])
            rhs_tiles.append(lks[:, 0, :])
            rhs_tiles.append(lks[:, 1, :])

            yt = hpool.tile([P, 2, BCH], BF16)
            for mt in range(2):
                hp = hppool.tile([P, BCH], FP32, tag="hp")
                for kt in range(10):
                    nc.tensor.matmul(
                        hp[:],
                        w1_sb[:, kt, mt, :],
                        rhs_tiles[kt],
                        start=(kt == 0),
                        stop=(kt == 9),
                    )
                nc.scalar.activation(
                    yt[:, mt, :], hp[:], AF.Relu, bias=b1_sb[:, mt : mt + 1]
                )

            op_ = hppool.tile([1, BCH], FP32, tag="op")
            for kt in range(2):
                nc.tensor.matmul(
                    op_[:],
                    wc_sb[:, kt : kt + 1],
                    yt[:, kt, :],
                    start=(kt == 0),
                    stop=(kt == 1),
                )
            res = hpool.tile([1, BCH], FP32)
            nc.scalar.activation(res[:], op_[:], AF.Sigmoid, bias=bc_sb[0:1, 0:1])
            nc.sync.dma_start(out[:], res[:])

    nc.compile()
    return nc


def prep_inputs(inputs, T=512, n_layers=NLAY):
    """Host-side: shard + pre-layout all tensors. Returns in_maps list."""
    ci = np.asarray(inputs["comment_indices"]).astype(np.int32)
    co = np.asarray(inputs["code_indices"]).astype(np.int32)
    emb_bf = np.ascontiguousarray(
        np.asarray(inputs["embed_table"], np.float32)
    ).astype(NP_BF16)
    gwx = np.asarray(inputs["gru_Wx"], np.float32)
    gwh = np.asarray(inputs["gru_Wh"], np.float32)
    gb = np.asarray(inputs["gru_b"], np.float32)

    wx_t = np.ascontiguousarray(
        gwx[:n_layers].reshape(n_layers, 2, JX, P, JG, P).transpose(0, 1, 2, 4, 3, 5)
    ).astype(NP_BF16)
    wh_t = np.ascontiguousarray(
        gwh[:n_layers].reshape(n_layers, 2, KH, P, JG, P).transpose(0, 1, 2, 4, 3, 5)
    ).astype(NP_BF16)

    pb = gb[:n_layers, :, 0, :].copy()  # [nl, 2, 768]
    pb[:, :, : 2 * H] += gb[:n_layers, :, 1, : 2 * H]
    pbias_h = np.ascontiguousarray(
        pb.reshape(n_layers, 2, JG, P).transpose(3, 0, 1, 2)
    ).astype(np.float32)
    b1row_h = np.ascontiguousarray(
        gb[:n_layers, :, 1, 2 * H :].reshape(1, n_layers, 2, JH, P)
    ).astype(NP_BF16)

    s1 = np.asarray(inputs["bn1_gamma"], np.float32) / np.sqrt(
        np.asarray(inputs["bn1_var"], np.float32) + EPS
    )
    t1 = (
        np.asarray(inputs["bn1_beta"], np.float32)
        - np.asarray(inputs["bn1_mean"], np.float32) * s1
    )
    fc1 = np.asarray(inputs["fc1_W"], np.float32)
    w1p = fc1 * s1[:, None]
    b1v = t1 @ fc1 + np.asarray(inputs["fc1_b"], np.float32)
    s2 = np.asarray(inputs["bn2_gamma"], np.float32) / np.sqrt(
        np.asarray(inputs["bn2_var"], np.float32) + EPS
    )
    t2 = (
        np.asarray(inputs["bn2_beta"], np.float32)
        - np.asarray(inputs["bn2_mean"], np.float32) * s2
    )
    clsw = np.asarray(inputs["cls_W"], np.float32)
    wcp = clsw * s2[:, None]
    bcp = (t2 @ clsw + np.asarray(inputs["cls_b"], np.float32)).reshape(1, 1)

    w1_t = np.ascontiguousarray(w1p.reshape(10, P, 2, P).transpose(0, 2, 1, 3)).astype(
        NP_BF16
    )
    b1p_h = np.ascontiguousarray(b1v.reshape(2, P).T).astype(np.float32)
    wc_h = np.ascontiguousarray(wcp.reshape(2, P).T).astype(NP_BF16)

    lw = np.asarray(inputs["leaks_W"], np.float32)
    lw0_h = np.ascontiguousarray(lw[:P].reshape(P, 2, P)).astype(NP_BF16)
    lw1_h = np.ascontiguousarray(lw[P:].reshape(20, 2, P)).astype(NP_BF16)
    lb_h = np.ascontiguousarray(
        np.asarray(inputs["leaks_b"], np.float32).reshape(2, P).T
    ).astype(np.float32)
    leaks = np.asarray(inputs["leaks_indices"], np.float32)

    shared = dict(
        emb=emb_bf, wx=wx_t, wh=wh_t, pbias=pbias_h, b1row=b1row_h,
        w1=w1_t, b1p=b1p_h, wc=wc_h, bc=bcp.astype(np.float32),
        lw0=lw0_h, lw1=lw1_h, lb=lb_h,
    )
    in_maps = []
    for c in range(NCORES):
        exs = slice(BCH * c, BCH * c + BCH)
        merged = np.concatenate([co[exs, :T], ci[exs, :T]], 0)  # [8, T]
        idxw_h = np.ascontiguousarray(
            merged.reshape(BC, T // P, P).transpose(2, 0, 1).reshape(P, -1)
        ).astype(np.int32)
        lkt = np.ascontiguousarray(leaks[exs].T).astype(NP_BF16)
        m = dict(shared)
        m["idxw"] = idxw_h
        m["leakst"] = lkt
        in_maps.append(m)
    return in_maps


def kernel(**inputs) -> np.ndarray:
    from concourse.bass_utils import run_bass_kernel_spmd

    nc = build_nc(T=512)
    in_maps = prep_inputs(inputs, T=512)
    res = run_bass_kernel_spmd(nc, in_maps, list(range(NCORES)))
    outs = [np.asarray(res.results[c]["out"]).reshape(-1) for c in range(NCORES)]
    return np.concatenate(outs).astype(np.float32)


if __name__ == "__main__":
    sys.path.insert(0, "/root/problem")
    import reference

    inp = {k: np.asarray(v) for k, v in reference.setup_inputs().items()}
    got = kernel(**inp)
    print("kernel out:", got[:8])


# revision 15
# speedup vs baseline: 1.1977x; 1.1500x over previous
"""Trainium2 Bass kernel for nn_BaselineModel_27298812133937.

Model: two [32,512] token sequences -> shared embedding [50000,512] ->
3 stacked bi-GRU layers (H=256, Keras reset_after) -> last states,
plus a leaks MLP branch, then BN/FC/BN/sigmoid head -> [32].

Sharding: the two sequences share GRU weights, so they merge into a
batch of 64. Each of the 8 cores takes 8 merged examples (4 code + 4
comment of the same original examples), runs the full network for its
shard with no cross-core communication, and computes the head for its
4 original examples. Host concatenates the 8x[4] outputs.

v3 scan dataflow (trace-driven): tiny matmuls issue every ~27ns on the
PE (the 167ns slice is pipeline latency), so PE work is cheap while the
DVE and the serial gate chain dominate. Per step the precomputed input
projection xz is accumulated into PSUM via an identity matmul, the
recurrent h-gate bias via a K=1 bias-row matmul, so the gate math is
down to sigmoid/tanh on the Act engine plus 5 tensor_tensor ops that
run on DVE for the fw direction and GpSimd for the bw direction. The
new state is written directly into a bf16 staging tile that both the
next step's matmul and the x_next DMA read.
"""

import os
import sys

import numpy as np

for _p in ("/opt/trn_rl_repo",):
    if os.path.isdir(_p) and _p not in sys.path:
        sys.path.insert(0, _p)

import concourse.bass as bass
import concourse.tile as tile
from concourse import bacc, mybir
from concourse.masks import make_identity

import ml_dtypes

FP32 = mybir.dt.float32
BF16 = mybir.dt.bfloat16
I32 = mybir.dt.int32
AF = mybir.ActivationFunctionType
OP = mybir.AluOpType
NP_BF16 = ml_dtypes.bfloat16

V, E, H, NLAY = 50000, 512, 256, 3
EPS = 1e-3
P = 128
JX = E // P        # 4  x-feature tiles
JG = 3 * H // P    # 6  gate tiles
JH = H // P        # 2  hidden tiles
KH = H // P        # 2  Wh contraction tiles
BC = 8             # merged examples per core
BCH = 4            # head (original) examples per core
NCORES = 8
U = 8              # scan unroll == xp time-block


def build_nc(T=512, n_layers=NLAY, use_for_i=True, staggered=True, debug=False):
    assert T % P == 0 and T % (2 * U) == 0
    TB = T // U
    TB2 = T // (2 * U)
    NCH = BC * (T // P)

    nc = bacc.Bacc("TRN2", target_bir_lowering=False, debug=debug)

    def din(name, shape, dt):
        return nc.declare_dram_parameter(name, list(shape), dt, False)

    emb = din("emb", [V, E], BF16)
    idxw = din("idxw", [P, NCH], I32)
    wx = din("wx", [n_layers, 2, JX, JG, P, P], BF16)
    wh = din("wh", [n_layers, 2, KH, JG, P, P], BF16)
    pbias = din("pbias", [P, n_layers, 2, JG], FP32)
    bbias = din("bbias", [P, n_layers, 2, JH, BC], BF16)
    w1 = din("w1", [10, 2, P, P], BF16)
    b1p = din("b1p", [P, 2], FP32)
    wc = din("wc", [P, 2], BF16)
    bc_b = din("bc", [1, 1], FP32)
    lw0 = din("lw0", [P, 2, P], BF16)
    lw1 = din("lw1", [20, 2, P], BF16)
    lb = din("lb", [P, 2], FP32)
    leakst = din("leakst", [148, BCH], BF16)

    out = nc.declare_dram_parameter("out", [1, BCH], FP32, True)

    # internal DRAM
    x_bufs = [nc.dram_tensor(f"x{i}", [P, JX, T, BC], BF16) for i in range(2)]
    # xp: merged projection (zr gates 0:4 with full bias; xh 4:6 with input
    # bias). bw (d=1) stored time-reversed so the scan indexes both dirs
    # identically.
    xp = [nc.dram_tensor(f"xp{d}", [P, TB, JG, BC, U], BF16) for d in range(2)]

    with tile.TileContext(nc) as tc, tc.tile_pool(name="const", bufs=1) as cpool:
        # ---- constants in SBUF
        ident = cpool.tile([P, P], BF16)
        make_identity(nc, ident[:])
        idx_sb = cpool.tile([P, NCH], I32)
        nc.sync.dma_start(idx_sb[:], idxw[:])
        pb_sb = cpool.tile([P, n_layers, 2, JG], FP32)
        nc.sync.dma_start(pb_sb[:], pbias[:])
        bb_sb = cpool.tile([P, n_layers, 2, JH, BC], BF16)
        nc.sync.dma_start(bb_sb[:], bbias[:])
        fin_hold = [cpool.tile([P, JH, BC], BF16, name=f"fin{i}") for i in range(2)]

        # ---- phase 1: embedding gather -> x0 (layer-0 input, transposed)
        x0 = x_bufs[0]
        with (
            tc.tile_pool(name="erow", bufs=3) as epool,
            tc.tile_pool(name="estage", bufs=2) as espool,
            tc.tile_pool(name="epsum", bufs=4, space="PSUM") as eppool,
        ):
            for tc_i in range(T // P):
                stages = [
                    espool.tile([P, P, BC], BF16, name=f"estg{j}", tag=f"st{j}")
                    for j in range(JX)
                ]
                for bi in range(BC):
                    ch = bi * (T // P) + tc_i
                    g = epool.tile([P, E], BF16)
                    nc.gpsimd.indirect_dma_start(
                        out=g[:],
                        out_offset=None,
                        in_=emb[:],
                        in_offset=bass.IndirectOffsetOnAxis(
                            ap=idx_sb[:, ch : ch + 1], axis=0
                        ),
                    )
                    for j in range(JX):
                        pst = eppool.tile([P, P], BF16)
                        nc.tensor.transpose(pst[:], g[:, j * P : (j + 1) * P], ident[:])
                        if (bi + j) % 2 == 0:
                            nc.vector.tensor_copy(stages[j][:, :, bi], pst[:])
                        else:
                            nc.scalar.copy(stages[j][:, :, bi], pst[:])
                for j in range(JX):
                    nc.sync.dma_start(
                        x0[:, j, tc_i * P : (tc_i + 1) * P, :], stages[j][:]
                    )

        # ---- per-layer: proj (both dirs) then scan (both dirs)
        for l in range(n_layers):
            x_cur = x_bufs[l % 2]
            x_next = x_bufs[(l + 1) % 2]
            is_last = l == n_layers - 1

            # -- input projection: xp^T = Wx^T @ x^T (+bias), to DRAM (bf16)
            with (
                tc.tile_pool(name="wts", bufs=1) as wpool,
                tc.tile_pool(name="pstage", bufs=3) as pspool,
                tc.tile_pool(name="ppsum", bufs=2, space="PSUM") as pppool,
                tc.tile_pool(name="xchunk", bufs=2) as xcpool,
            ):
                wx_sb = wpool.tile([P, 2, JX, JG, P], BF16)
                nc.sync.dma_start(wx_sb[:], wx[l].rearrange("d kt mt p q -> p d kt mt q"))

                NCK = T // 64  # chunks of 512 cols (64 t x 8 b)
                for d in range(2):
                    for c in range(NCK):
                        xch = xcpool.tile([P, JX, 64, BC], BF16, tag="xch")
                        nc.sync.dma_start(xch[:], x_cur[:, :, c * 64 : (c + 1) * 64, :])
                        for mt in range(JG):
                            ps = pppool.tile([P, 512], FP32)
                            for kt in range(JX):
                                nc.tensor.matmul(
                                    ps[:],
                                    wx_sb[:, d, kt, mt, :],
                                    xch[:, kt, :, :],
                                    start=(kt == 0),
                                    stop=(kt == JX - 1),
                                )
                            # staging layout [P, tb, b, ti]; psum iter order is
                            # (t, b) = (tb, ti, b) -> permuted write AP
                            stg = pspool.tile([P, 8, BC, U], BF16, tag="stg")
                            if d == 0:
                                stg_w = stg[:].rearrange("p tb b ti -> p tb ti b")
                                tb_lo = c * 8
                                dst_tb = slice(tb_lo, tb_lo + 8)
                            else:
                                # bw: store reversed in time (block and
                                # within-block order both reversed) by writing
                                # the staging tile through a reversed AP
                                stg_w = stg[:, ::-1, :, ::-1].rearrange(
                                    "p tb b ti -> p tb ti b"
                                )
                                tb_hi = TB - c * 8
                                dst_tb = slice(tb_hi - 8, tb_hi)
                            if mt % 2 == 0:
                                nc.vector.tensor_scalar_add(
                                    stg_w, ps[:], pb_sb[:, l, d, mt : mt + 1]
                                )
                            else:
                                nc.scalar.activation(
                                    stg_w,
                                    ps[:],
                                    AF.Identity,
                                    bias=pb_sb[:, l, d, mt : mt + 1],
                                )
                            nc.sync.dma_start(xp[d][:, dst_tb, mt, :, :], stg[:])

            # -- scan
            with (
                tc.tile_pool(name="state", bufs=4) as stpool,
                tc.tile_pool(name="gates", bufs=3) as gpool,
                tc.tile_pool(name="xpchunk", bufs=3) as xppool,
                tc.tile_pool(name="spsum", bufs=2, space="PSUM") as sppool,
                tc.tile_pool(name="wts2", bufs=1) as wpool2,
            ):
                wh_sb = wpool2.tile([P, 2, KH, JG, P], BF16)
                nc.sync.dma_start(wh_sb[:], wh[l].rearrange("d kt mt p q -> p d kt mt q"))

                # fixed staging tiles: state slots double as x_next staging.
                # body handles 2 sub-blocks (A then B); A's u=0 reads B's
                # last slot from the previous iteration.
                stg = []  # [d][sub] -> tile [P, JH, U, BC] bf16
                for d in range(2):
                    sA = stpool.tile([P, JH, U, BC], BF16, name=f"stgA{d}")
                    sB = stpool.tile([P, JH, U, BC], BF16, name=f"stgB{d}")
                    nc.vector.memset(sB[:], 0.0)
                    stg.append((sA, sB))

                def scan_sub(ib, sub, dyn):
                    # chunk loads (one per dir): [P, JG, BC, U] bf16
                    chunks = []
                    for d in range(2):
                        if dyn:
                            tbs = bass.ds(ib * 2 + sub, 1)
                        else:
                            tbs = slice(ib * 2 + sub, ib * 2 + sub + 1)
                        xc = xppool.tile([P, 1, JG, BC, U], BF16, tag=f"xc{d}{sub}")
                        nc.sync.dma_start(xc[:], xp[d][:, tbs, :, :, :])
                        chunks.append(xc)
                    cur = [stg[d][sub] for d in range(2)]
                    prev = [stg[d][1 - sub] for d in range(2)]
                    for u in range(U):
                        vprevs, slots = [], []
                        for d in range(2):
                            # fw fills slots ascending (slot u == time base+u);
                            # bw fills descending (slot U-1-u) so the tile ends
                            # up in ascending-time order for a direct DMA.
                            if d == 0:
                                slots.append(u)
                                vprevs.append(
                                    cur[d][:, :, u - 1, :]
                                    if u > 0
                                    else prev[d][:, :, U - 1, :]
                                )
                            else:
                                slots.append(U - 1 - u)
                                vprevs.append(
                                    cur[d][:, :, U - u, :]
                                    if u > 0
                                    else prev[d][:, :, 0, :]
                                )
                        # critical r-gate psum first for both dirs (5 mms each),
                        # then the fat z+h groups; all stationaries are K=128 so
                        # the PE never switches tile geometry.
                        ptr_, ptzh_ = [], []
                        for d in range(2):
                            xz_r = chunks[d][:, 0, 2:4, :, u]
                            ptr = sppool.tile([P, 2, BC], FP32, tag=f"pr{d}")
                            nc.tensor.matmul(
                                ptr[:], ident[:], xz_r, start=True, stop=False,
                                skip_group_check=True,
                            )
                            for mt in range(2):
                                for kt in range(KH):
                                    nc.tensor.matmul(
                                        ptr[:, mt, :],
                                        wh_sb[:, d, kt, 2 + mt, :],
                                        vprevs[d][:, kt, :],
                                        start=False, stop=(kt == KH - 1),
                                        skip_group_check=True,
                                    )
                            ptr_.append(ptr)
                        for d in range(2):
                            xz_z = chunks[d][:, 0, 0:2, :, u]
                            pzh = sppool.tile([P, 4, BC], FP32, tag=f"pzh{d}")
                            nc.tensor.matmul(
                                pzh[:, 0:2, :], ident[:], xz_z,
                                start=True, stop=False, skip_group_check=True,
                            )
                            nc.tensor.matmul(
                                pzh[:, 2:4, :], ident[:], bb_sb[:, l, d, :, :],
                                start=True, stop=False, skip_group_check=True,
                            )
                            for mt in range(2):
                                for kt in range(KH):
                                    nc.tensor.matmul(
                                        pzh[:, mt, :],
                                        wh_sb[:, d, kt, mt, :],
                                        vprevs[d][:, kt, :],
                                        start=False, stop=(kt == KH - 1),
                                        skip_group_check=True,
                                    )
                            for jh in range(JH):
                                for kt in range(KH):
                                    nc.tensor.matmul(
                                        pzh[:, 2 + jh, :],
                                        wh_sb[:, d, kt, 4 + jh, :],
                                        vprevs[d][:, kt, :],
                                        start=False, stop=(kt == KH - 1),
                                        skip_group_check=True,
                                    )
                            ptzh_.append(pzh)
                        for d in range(2):
                            vprev, slot_w = vprevs[d], slots[d]
                            xh = chunks[d][:, 0, 4:6, :, u]
                            # gpsimd cannot touch PSUM: hm always on DVE,
                            # the rest of the bw chain on gpsimd.
                            eng = nc.vector if d == 0 else nc.gpsimd
                            r_ = gpool.tile([P, 2, BC], FP32, tag=f"r{d}")
                            nc.scalar.activation(r_[:], ptr_[d][:], AF.Sigmoid)
                            z_ = gpool.tile([P, 2, BC], FP32, tag=f"z{d}")
                            nc.scalar.activation(
                                z_[:], ptzh_[d][:, 0:2, :], AF.Sigmoid
                            )
                            # off-chain helpers: w = z*vprev, zm1 = z-1
                            w_ = gpool.tile([P, 2, BC], FP32, tag=f"w{d}")
                            eng.tensor_tensor(w_[:], z_[:], vprev, OP.mult)
                            zm1 = gpool.tile([P, 2, BC], FP32, tag=f"zm{d}")
                            eng.tensor_scalar_add(zm1[:], z_[:], -1.0)
                            # critical tail
                            hm = gpool.tile([P, 2, BC], FP32, tag=f"hm{d}")
                            nc.vector.tensor_tensor(
                                hm[:], ptzh_[d][:, 2:4, :], r_[:], OP.mult
                            )
                            av = gpool.tile([P, 2, BC], FP32, tag=f"av{d}")
                            eng.tensor_tensor(av[:], hm[:], xh, OP.add)
                            hh = gpool.tile([P, 2, BC], FP32, tag=f"hh{d}")
                            nc.scalar.activation(hh[:], av[:], AF.Tanh)
                            g_ = gpool.tile([P, 2, BC], FP32, tag=f"g{d}")
                            eng.tensor_tensor(g_[:], zm1[:], hh[:], OP.mult)
                            # v = w - g = z*vprev + (1-z)*hh
                            eng.tensor_tensor(
                                cur[d][:, :, slot_w, :], w_[:], g_[:], OP.subtract
                            )
                    if not is_last:
                        for d in range(2):
                            if d == 0:
                                if dyn:
                                    t_ap = bass.ds(ib * (2 * U) + sub * U, U)
                                else:
                                    t0 = ib * 2 * U + sub * U
                                    t_ap = slice(t0, t0 + U)
                            else:
                                if dyn:
                                    t_ap = bass.ds(
                                        ib * (-2 * U) + (T - U - sub * U), U
                                    )
                                else:
                                    t0 = T - U - sub * U - ib * 2 * U
                                    t_ap = slice(t0, t0 + U)
                            nc.sync.dma_start(
                                x_next[:, 2 * d : 2 * d + 2, t_ap, :], cur[d][:]
                            )

                if use_for_i:
                    with tc.For_i(0, TB2, 1, staggered_reset=staggered) as ib:
                        scan_sub(ib, 0, True)
                        scan_sub(ib, 1, True)
                else:
                    for ib in range(TB2):
                        scan_sub(ib, 0, False)
                        scan_sub(ib, 1, False)

                if is_last:
                    # final state: fw in last slot, bw in slot 0 (descending fill)
                    nc.vector.tensor_copy(fin_hold[0][:], stg[0][1][:, :, U - 1, :])
                    nc.vector.tensor_copy(fin_hold[1][:], stg[1][1][:, :, 0, :])

        # ---- head: leaks branch + folded BN/FC/BN/sigmoid
        with (
            tc.tile_pool(name="head", bufs=1) as hpool,
            tc.tile_pool(name="hpsum", bufs=2, space="PSUM") as hppool,
        ):
            lkw0 = hpool.tile([P, 2, P], BF16)
            nc.sync.dma_start(lkw0[:], lw0[:])
            lkw1 = hpool.tile([20, 2, P], BF16)
            nc.sync.dma_start(lkw1[:], lw1[:])
            lkb = hpool.tile([P, 2], FP32)
            nc.sync.dma_start(lkb[:], lb[:])
            lkx0 = hpool.tile([P, BCH], BF16)
            nc.sync.dma_start(lkx0[:], leakst[0:P, :])
            lkx1 = hpool.tile([20, BCH], BF16)
            nc.sync.dma_start(lkx1[:], leakst[P:148, :])

            lks = hpool.tile([P, 2, BCH], BF16)
            for mt in range(2):
                lp = hppool.tile([P, BCH], FP32, tag="lp")
                nc.tensor.matmul(lp[:], lkw0[:, mt, :], lkx0[:], start=True, stop=False)
                nc.tensor.matmul(lp[:], lkw1[:, mt, :], lkx1[:], start=False, stop=True)
                nc.scalar.activation(
                    lks[:, mt, :], lp[:], AF.Relu, bias=lkb[:, mt : mt + 1]
                )

            w1_sb = hpool.tile([P, 10, 2, P], BF16)
            nc.sync.dma_start(w1_sb[:], w1[:].rearrange("kt mt p q -> p kt mt q"))
            b1_sb = hpool.tile([P, 2], FP32)
            nc.sync.dma_start(b1_sb[:], b1p[:])
            wc_sb = hpool.tile([P, 2], BF16)
            nc.sync.dma_start(wc_sb[:], wc[:])
            bc_sb = hpool.tile([1, 1], FP32)
            nc.sync.dma_start(bc_sb[:], bc_b[:])

            sf, sb_ = fin_hold
            rhs_tiles = []
            for half in range(2):  # code (cols 0:4), comment (cols 4:8)
                c0 = half * BCH
                for dstate in (sf, sb_):
                    for j in range(JH):
                        rhs_tiles.append(dstate[:, j, c0 : c0 + BCH])
            rhs_tiles.append(lks[:, 0, :])
            rhs_tiles.append(lks[:, 1, :])

            yt = hpool.tile([P, 2, BCH], BF16)
            for mt in range(2):
                hp = hppool.tile([P, BCH], FP32, tag="hp")
                for kt in range(10):
                    nc.tensor.matmul(
                        hp[:],
                        w1_sb[:, kt, mt, :],
                        rhs_tiles[kt],
                        start=(kt == 0),
                        stop=(kt == 9),
                    )
                nc.scalar.activation(
                    yt[:, mt, :], hp[:], AF.Relu, bias=b1_sb[:, mt : mt + 1]
                )

            op_ = hppool.tile([1, BCH], FP32, tag="op")
            for kt in range(2):
                nc.tensor.matmul(
                    op_[:],
                    wc_sb[:, kt : kt + 1],
                    yt[:, kt, :],
                    start=(kt == 0),
                    stop=(kt == 1),
                )
            res = hpool.tile([1, BCH], FP32)
            nc.scalar.activation(res[:], op_[:], AF.Sigmoid, bias=bc_sb[0:1, 0:1])
            nc.sync.dma_start(out[:], res[:])

    nc.compile()
    return nc


def prep_inputs(inputs, T=512, n_layers=NLAY):
    """Host-side: shard + pre-layout all tensors. Returns in_maps list."""
    ci = np.asarray(inputs["comment_indices"]).astype(np.int32)
    co = np.asarray(inputs["code_indices"]).astype(np.int32)
    emb_bf = np.ascontiguousarray(
        np.asarray(inputs["embed_table"], np.float32)
    ).astype(NP_BF16)
    gwx = np.asarray(inputs["gru_Wx"], np.float32)
    gwh = np.asarray(inputs["gru_Wh"], np.float32)
    gb = np.asarray(inputs["gru_b"], np.float32)

    wx_t = np.ascontiguousarray(
        gwx[:n_layers].reshape(n_layers, 2, JX, P, JG, P).transpose(0, 1, 2, 4, 3, 5)
    ).astype(NP_BF16)
    wh_t = np.ascontiguousarray(
        gwh[:n_layers].reshape(n_layers, 2, KH, P, JG, P).transpose(0, 1, 2, 4, 3, 5)
    ).astype(NP_BF16)

    pb = gb[:n_layers, :, 0, :].copy()  # [nl, 2, 768]
    pb[:, :, : 2 * H] += gb[:n_layers, :, 1, : 2 * H]
    pbias_h = np.ascontiguousarray(
        pb.reshape(n_layers, 2, JG, P).transpose(3, 0, 1, 2)
    ).astype(np.float32)
    # recurrent h-gate bias broadcast along batch: [P, nl, 2, JH, BC]
    b1h = gb[:n_layers, :, 1, 2 * H :].reshape(n_layers, 2, JH, P)
    bbias_h = np.ascontiguousarray(
        np.broadcast_to(
            b1h.transpose(3, 0, 1, 2)[:, :, :, :, None], (P, n_layers, 2, JH, BC)
        )
    ).astype(NP_BF16)

    s1 = np.asarray(inputs["bn1_gamma"], np.float32) / np.sqrt(
        np.asarray(inputs["bn1_var"], np.float32) + EPS
    )
    t1 = (
        np.asarray(inputs["bn1_beta"], np.float32)
        - np.asarray(inputs["bn1_mean"], np.float32) * s1
    )
    fc1 = np.asarray(inputs["fc1_W"], np.float32)
    w1p = fc1 * s1[:, None]
    b1v = t1 @ fc1 + np.asarray(inputs["fc1_b"], np.float32)
    s2 = np.asarray(inputs["bn2_gamma"], np.float32) / np.sqrt(
        np.asarray(inputs["bn2_var"], np.float32) + EPS
    )
    t2 = (
        np.asarray(inputs["bn2_beta"], np.float32)
        - np.asarray(inputs["bn2_mean"], np.float32) * s2
    )
    clsw = np.asarray(inputs["cls_W"], np.float32)
    wcp = clsw * s2[:, None]
    bcp = (t2 @ clsw + np.asarray(inputs["cls_b"], np.float32)).reshape(1, 1)

    w1_t = np.ascontiguousarray(w1p.reshape(10, P, 2, P).transpose(0, 2, 1, 3)).astype(
        NP_BF16
    )
    b1p_h = np.ascontiguousarray(b1v.reshape(2, P).T).astype(np.float32)
    wc_h = np.ascontiguousarray(wcp.reshape(2, P).T).astype(NP_BF16)

    lw = np.asarray(inputs["leaks_W"], np.float32)
    lw0_h = np.ascontiguousarray(lw[:P].reshape(P, 2, P)).astype(NP_BF16)
    lw1_h = np.ascontiguousarray(lw[P:].reshape(20, 2, P)).astype(NP_BF16)
    lb_h = np.ascontiguousarray(
        np.asarray(inputs["leaks_b"], np.float32).reshape(2, P).T
    ).astype(np.float32)
    leaks = np.asarray(inputs["leaks_indices"], np.float32)

    shared = dict(
        emb=emb_bf, wx=wx_t, wh=wh_t, pbias=pbias_h, bbias=bbias_h,
        w1=w1_t, b1p=b1p_h, wc=wc_h, bc=bcp.astype(np.float32),
        lw0=lw0_h, lw1=lw1_h, lb=lb_h,
    )
    in_maps = []
    for c in range(NCORES):
        exs = slice(BCH * c, BCH * c + BCH)
        merged = np.concatenate([co[exs, :T], ci[exs, :T]], 0)  # [8, T]
        idxw_h = np.ascontiguousarray(
            merged.reshape(BC, T // P, P).transpose(2, 0, 1).reshape(P, -1)
        ).astype(np.int32)
        lkt = np.ascontiguousarray(leaks[exs].T).astype(NP_BF16)
        m = dict(shared)
        m["idxw"] = idxw_h
        m["leakst"] = lkt
        in_maps.append(m)
    return in_maps


def kernel(**inputs) -> np.ndarray:
    from concourse.bass_utils import run_bass_kernel_spmd

    nc = build_nc(T=512)
    in_maps = prep_inputs(inputs, T=512)
    res = run_bass_kernel_spmd(nc, in_maps, list(range(NCORES)))
    outs = [np.asarray(res.results[c]["out"]).reshape(-1) for c in range(NCORES)]
    return np.concatenate(outs).astype(np.float32)


if __name__ == "__main__":
    sys.path.insert(0, "/root/problem")
    import reference

    inp = {k: np.asarray(v) for k, v in reference.setup_inputs().items()}
    got = kernel(**inp)
    print("kernel out:", got[:8])


# revision 19
# speedup vs baseline: 1.3726x; 1.1460x over previous
"""Trainium2 Bass kernel for nn_BaselineModel_27298812133937.

Model: two [32,512] token sequences -> shared embedding [50000,512] ->
3 stacked bi-GRU layers (H=256, Keras reset_after) -> last states,
plus a leaks MLP branch, then BN/FC/BN/sigmoid head -> [32].

Sharding: the two sequences share GRU weights, so they merge into a
batch of 64. Each of the 8 cores takes 8 merged examples (4 code + 4
comment of the same original examples), runs the full network for its
shard with no cross-core communication, and computes the head for its
4 original examples. Host concatenates the 8x[4] outputs.

v3 scan dataflow (trace-driven): tiny matmuls issue every ~27ns on the
PE (the 167ns slice is pipeline latency), so PE work is cheap while the
DVE and the serial gate chain dominate. Per step the precomputed input
projection xz is accumulated into PSUM via an identity matmul, the
recurrent h-gate bias via a K=1 bias-row matmul, so the gate math is
down to sigmoid/tanh on the Act engine plus 5 tensor_tensor ops that
run on DVE for the fw direction and GpSimd for the bw direction. The
new state is written directly into a bf16 staging tile that both the
next step's matmul and the x_next DMA read.
"""

import os
import sys

import numpy as np

for _p in ("/opt/trn_rl_repo",):
    if os.path.isdir(_p) and _p not in sys.path:
        sys.path.insert(0, _p)

import concourse.bass as bass
import concourse.tile as tile
from concourse import bacc, mybir
from concourse.masks import make_identity

import ml_dtypes

FP32 = mybir.dt.float32
BF16 = mybir.dt.bfloat16
I32 = mybir.dt.int32
AF = mybir.ActivationFunctionType
OP = mybir.AluOpType
NP_BF16 = ml_dtypes.bfloat16

V, E, H, NLAY = 50000, 512, 256, 3
EPS = 1e-3
P = 128
JX = E // P        # 4  x-feature tiles
JG = 3 * H // P    # 6  gate tiles
JH = H // P        # 2  hidden tiles
KH = H // P        # 2  Wh contraction tiles
BC = 8             # merged examples per core
BCH = 4            # head (original) examples per core
NCORES = 8
U = 8              # scan unroll == xp time-block


def build_nc(T=512, n_layers=NLAY, use_for_i=True, staggered=True, debug=False):
    assert T % P == 0 and T % (2 * U) == 0
    TB = T // U
    TB2 = T // (2 * U)
    NCH = BC * (T // P)

    nc = bacc.Bacc("TRN2", target_bir_lowering=False, debug=debug)

    def din(name, shape, dt):
        return nc.declare_dram_parameter(name, list(shape), dt, False)

    emb = din("emb", [V, E], BF16)
    idxw = din("idxw", [P, NCH], I32)
    wx = din("wx", [n_layers, 2, JX, JG, P, P], BF16)
    wh = din("wh", [n_layers, 2, KH, JG, P, P], BF16)
    pbias = din("pbias", [P, n_layers, 2, JG], FP32)
    bbias = din("bbias", [P, n_layers, 2, JH, BC], BF16)
    w1 = din("w1", [10, 2, P, P], BF16)
    b1p = din("b1p", [P, 2], FP32)
    wc = din("wc", [P, 2], BF16)
    bc_b = din("bc", [1, 1], FP32)
    lw0 = din("lw0", [P, 2, P], BF16)
    lw1 = din("lw1", [20, 2, P], BF16)
    lb = din("lb", [P, 2], FP32)
    leakst = din("leakst", [148, BCH], BF16)

    out = nc.declare_dram_parameter("out", [1, BCH], FP32, True)

    # internal DRAM
    x_bufs = [nc.dram_tensor(f"x{i}", [P, JX, T, BC], BF16) for i in range(2)]
    # xp: merged projection (zr gates 0:4 with full bias; xh 4:6 with input
    # bias). bw (d=1) stored time-reversed so the scan indexes both dirs
    # identically.
    xp = [nc.dram_tensor(f"xp{d}", [P, TB, JG, BC, U], BF16) for d in range(2)]

    with tile.TileContext(nc) as tc, tc.tile_pool(name="const", bufs=1) as cpool:
        # ---- constants in SBUF
        ident = cpool.tile([P, P], BF16)
        make_identity(nc, ident[:])
        idx_sb = cpool.tile([P, NCH], I32)
        nc.sync.dma_start(idx_sb[:], idxw[:])
        pb_sb = cpool.tile([P, n_layers, 2, JG], FP32)
        nc.sync.dma_start(pb_sb[:], pbias[:])
        bb_sb = cpool.tile([P, n_layers, 2, JH, BC], BF16)
        nc.sync.dma_start(bb_sb[:], bbias[:])
        fin_hold = [cpool.tile([P, JH, BC], BF16, name=f"fin{i}") for i in range(2)]

        # ---- phase 1: embedding gather -> x0 (layer-0 input, transposed)
        x0 = x_bufs[0]
        with (
            tc.tile_pool(name="erow", bufs=6) as epool,
            tc.tile_pool(name="estage", bufs=2) as espool,
            tc.tile_pool(name="epsum", bufs=8, space="PSUM") as eppool,
        ):
            for tc_i in range(T // P):
                stages = [
                    espool.tile([P, P, BC], BF16, name=f"estg{j}", tag=f"st{j}")
                    for j in range(JX)
                ]
                for bi in range(BC):
                    ch = bi * (T // P) + tc_i
                    g = epool.tile([P, E], BF16)
                    nc.gpsimd.indirect_dma_start(
                        out=g[:],
                        out_offset=None,
                        in_=emb[:],
                        in_offset=bass.IndirectOffsetOnAxis(
                            ap=idx_sb[:, ch : ch + 1], axis=0
                        ),
                    )
                    for j in range(JX):
                        pst = eppool.tile([P, P], BF16)
                        nc.tensor.transpose(pst[:], g[:, j * P : (j + 1) * P], ident[:])
                        if (bi + j) % 2 == 0:
                            nc.vector.tensor_copy(stages[j][:, :, bi], pst[:])
                        else:
                            nc.scalar.copy(stages[j][:, :, bi], pst[:])
                for j in range(JX):
                    nc.sync.dma_start(
                        x0[:, j, tc_i * P : (tc_i + 1) * P, :], stages[j][:]
                    )

        # ---- per-layer: proj (both dirs) then scan (both dirs)
        for l in range(n_layers):
            x_cur = x_bufs[l % 2]
            x_next = x_bufs[(l + 1) % 2]
            is_last = l == n_layers - 1

            # -- input projection: xp^T = Wx^T @ x^T (+bias), to DRAM (bf16)
            with (
                tc.tile_pool(name="wts", bufs=1) as wpool,
                tc.tile_pool(name="pstage", bufs=3) as pspool,
                tc.tile_pool(name="ppsum", bufs=2, space="PSUM") as pppool,
                tc.tile_pool(name="xchunk", bufs=2) as xcpool,
            ):
                wx_sb = wpool.tile([P, 2, JX, JG, P], BF16)
                nc.sync.dma_start(wx_sb[:], wx[l].rearrange("d kt mt p q -> p d kt mt q"))

                NCK = T // 64  # chunks of 512 cols (64 t x 8 b)
                for d in range(2):
                    for c in range(NCK):
                        xch = xcpool.tile([P, JX, 64, BC], BF16, tag="xch")
                        nc.sync.dma_start(xch[:], x_cur[:, :, c * 64 : (c + 1) * 64, :])
                        for mt in range(JG):
                            ps = pppool.tile([P, 512], FP32)
                            for kt in range(JX):
                                nc.tensor.matmul(
                                    ps[:],
                                    wx_sb[:, d, kt, mt, :],
                                    xch[:, kt, :, :],
                                    start=(kt == 0),
                                    stop=(kt == JX - 1),
                                )
                            # staging layout [P, tb, b, ti]; psum iter order is
                            # (t, b) = (tb, ti, b) -> permuted write AP
                            stg = pspool.tile([P, 8, BC, U], BF16, tag="stg")
                            if d == 0:
                                stg_w = stg[:].rearrange("p tb b ti -> p tb ti b")
                                tb_lo = c * 8
                                dst_tb = slice(tb_lo, tb_lo + 8)
                            else:
                                # bw: store reversed in time (block and
                                # within-block order both reversed) by writing
                                # the staging tile through a reversed AP
                                stg_w = stg[:, ::-1, :, ::-1].rearrange(
                                    "p tb b ti -> p tb ti b"
                                )
                                tb_hi = TB - c * 8
                                dst_tb = slice(tb_hi - 8, tb_hi)
                            if mt % 2 == 0:
                                nc.vector.tensor_scalar_add(
                                    stg_w, ps[:], pb_sb[:, l, d, mt : mt + 1]
                                )
                            else:
                                nc.scalar.activation(
                                    stg_w,
                                    ps[:],
                                    AF.Identity,
                                    bias=pb_sb[:, l, d, mt : mt + 1],
                                )
                            # xp slot order: r gates (mt 2,3) first, then z
                            # (mt 0,1), then h (mt 4,5) — the scan reads
                            # contiguous [0:4] for the rz psum and [4:6] for xh.
                            pos = {2: 0, 3: 1, 0: 2, 1: 3, 4: 4, 5: 5}[mt]
                            nc.sync.dma_start(xp[d][:, dst_tb, pos, :, :], stg[:])

            # -- scan
            with (
                tc.tile_pool(name="state", bufs=4) as stpool,
                tc.tile_pool(name="gates", bufs=4) as gpool,
                tc.tile_pool(name="xpchunk", bufs=3) as xppool,
                tc.tile_pool(name="spsum", bufs=2, space="PSUM") as sppool,
                tc.tile_pool(name="wts2", bufs=1) as wpool2,
            ):
                wh_sb = wpool2.tile([P, 2, KH, JG, P], BF16)
                nc.sync.dma_start(wh_sb[:], wh[l].rearrange("d kt mt p q -> p d kt mt q"))

                # fixed staging tiles: state slots double as x_next staging.
                # body handles 2 sub-blocks (A then B); A's u=0 reads B's
                # last slot from the previous iteration.
                stg = []  # [d][sub] -> tile [P, JH, U, BC] bf16
                for d in range(2):
                    sA = stpool.tile([P, JH, U, BC], BF16, name=f"stgA{d}")
                    sB = stpool.tile([P, JH, U, BC], BF16, name=f"stgB{d}")
                    nc.vector.memset(sB[:], 0.0)
                    stg.append((sA, sB))

                def scan_sub(ib, sub, dyn):
                    # chunk loads (one per dir): [P, JG, BC, U] bf16
                    chunks = []
                    for d in range(2):
                        if dyn:
                            tbs = bass.ds(ib * 2 + sub, 1)
                        else:
                            tbs = slice(ib * 2 + sub, ib * 2 + sub + 1)
                        xc = xppool.tile([P, 1, JG, BC, U], BF16, tag=f"xc{d}{sub}")
                        nc.sync.dma_start(xc[:], xp[d][:, tbs, :, :, :])
                        chunks.append(xc)
                    cur = [stg[d][sub] for d in range(2)]
                    prev = [stg[d][1 - sub] for d in range(2)]
                    for u in range(U):
                        vprevs, slots = [], []
                        for d in range(2):
                            # fw fills slots ascending (slot u == time base+u);
                            # bw fills descending (slot U-1-u) so the tile ends
                            # up in ascending-time order for a direct DMA.
                            if d == 0:
                                slots.append(u)
                                vprevs.append(
                                    cur[d][:, :, u - 1, :]
                                    if u > 0
                                    else prev[d][:, :, U - 1, :]
                                )
                            else:
                                slots.append(U - 1 - u)
                                vprevs.append(
                                    cur[d][:, :, U - u, :]
                                    if u > 0
                                    else prev[d][:, :, 0, :]
                                )
                        # rz psum (one wide identity + 8 wh mms, r-tiles first),
                        # then the h psum (identity-bias + 4 wh); all
                        # stationaries are K=128 so the PE never switches tile
                        # geometry.
                        ptrz_, pth_ = [], []
                        for d in range(2):
                            # [0:2] = r gates (mt 2,3), [2:4] = z gates (mt 0,1)
                            xz_rz = chunks[d][:, 0, 0:4, :, u]
                            prz = sppool.tile([P, 4, BC], FP32, tag=f"prz{d}")
                            nc.tensor.matmul(
                                prz[:], ident[:], xz_rz, start=True, stop=False,
                                skip_group_check=True,
                            )
                            for i, mt in enumerate((2, 3, 0, 1)):
                                for kt in range(KH):
                                    nc.tensor.matmul(
                                        prz[:, i, :],
                                        wh_sb[:, d, kt, mt, :],
                                        vprevs[d][:, kt, :],
                                        start=False, stop=(kt == KH - 1),
                                        skip_group_check=True,
                                    )
                            ptrz_.append(prz)
                        for d in range(2):
                            pth = sppool.tile([P, 2, BC], FP32, tag=f"ph{d}")
                            nc.tensor.matmul(
                                pth[:], ident[:], bb_sb[:, l, d, :, :],
                                start=True, stop=False, skip_group_check=True,
                            )
                            for jh in range(JH):
                                for kt in range(KH):
                                    nc.tensor.matmul(
                                        pth[:, jh, :],
                                        wh_sb[:, d, kt, 4 + jh, :],
                                        vprevs[d][:, kt, :],
                                        start=False, stop=(kt == KH - 1),
                                        skip_group_check=True,
                                    )
                            pth_.append(pth)
                        for d in range(2):
                            vprev, slot_w = vprevs[d], slots[d]
                            xh = chunks[d][:, 0, 4:6, :, u]
                            # gpsimd cannot touch PSUM and is slow on
                            # tensor_scalar forms: hm and g always on DVE, the
                            # rest of the bw chain on gpsimd.
                            eng = nc.vector if d == 0 else nc.gpsimd
                            rz = gpool.tile([P, 4, BC], FP32, tag=f"rz{d}")
                            nc.scalar.activation(rz[:], ptrz_[d][:], AF.Sigmoid)
                            # off-chain helper: w = z*vprev
                            w_ = gpool.tile([P, 2, BC], FP32, tag=f"w{d}")
                            eng.tensor_tensor(w_[:], rz[:, 2:4, :], vprev, OP.mult)
                            # critical tail
                            hm = gpool.tile([P, 2, BC], FP32, tag=f"hm{d}")
                            nc.vector.tensor_tensor(
                                hm[:], pth_[d][:], rz[:, 0:2, :], OP.mult
                            )
                            av = gpool.tile([P, 2, BC], FP32, tag=f"av{d}")
                            eng.tensor_tensor(av[:], hm[:], xh, OP.add)
                            hh = gpool.tile([P, 2, BC], FP32, tag=f"hh{d}")
                            nc.scalar.activation(hh[:], av[:], AF.Tanh)
                            # g = (z-1)*hh in one stt on DVE
                            g_ = gpool.tile([P, 2, BC], FP32, tag=f"g{d}")
                            nc.vector.scalar_tensor_tensor(
                                g_[:], rz[:, 2:4, :], -1.0, hh[:], OP.add, OP.mult
                            )
                            # v = w - g = z*vprev + (1-z)*hh
                            eng.tensor_tensor(
                                cur[d][:, :, slot_w, :], w_[:], g_[:], OP.subtract
                            )
                    if not is_last:
                        for d in range(2):
                            if d == 0:
                                if dyn:
                                    t_ap = bass.ds(ib * (2 * U) + sub * U, U)
                                else:
                                    t0 = ib * 2 * U + sub * U
                                    t_ap = slice(t0, t0 + U)
                            else:
                                if dyn:
                                    t_ap = bass.ds(
                                        ib * (-2 * U) + (T - U - sub * U), U
                                    )
                                else:
                                    t0 = T - U - sub * U - ib * 2 * U
                                    t_ap = slice(t0, t0 + U)
                            nc.sync.dma_start(
                                x_next[:, 2 * d : 2 * d + 2, t_ap, :], cur[d][:]
                            )

                if use_for_i:
                    with tc.For_i(0, TB2, 1, staggered_reset=staggered) as ib:
                        scan_sub(ib, 0, True)
                        scan_sub(ib, 1, True)
                else:
                    for ib in range(TB2):
                        scan_sub(ib, 0, False)
                        scan_sub(ib, 1, False)

                if is_last:
                    # final state: fw in last slot, bw in slot 0 (descending fill)
                    nc.vector.tensor_copy(fin_hold[0][:], stg[0][1][:, :, U - 1, :])
                    nc.vector.tensor_copy(fin_hold[1][:], stg[1][1][:, :, 0, :])

        # ---- head: leaks branch + folded BN/FC/BN/sigmoid
        with (
            tc.tile_pool(name="head", bufs=1) as hpool,
            tc.tile_pool(name="hpsum", bufs=2, space="PSUM") as hppool,
        ):
            lkw0 = hpool.tile([P, 2, P], BF16)
            nc.sync.dma_start(lkw0[:], lw0[:])
            lkw1 = hpool.tile([20, 2, P], BF16)
            nc.sync.dma_start(lkw1[:], lw1[:])
            lkb = hpool.tile([P, 2], FP32)
            nc.sync.dma_start(lkb[:], lb[:])
            lkx0 = hpool.tile([P, BCH], BF16)
            nc.sync.dma_start(lkx0[:], leakst[0:P, :])
            lkx1 = hpool.tile([20, BCH], BF16)
            nc.sync.dma_start(lkx1[:], leakst[P:148, :])

            lks = hpool.tile([P, 2, BCH], BF16)
            for mt in range(2):
                lp = hppool.tile([P, BCH], FP32, tag="lp")
                nc.tensor.matmul(lp[:], lkw0[:, mt, :], lkx0[:], start=True, stop=False)
                nc.tensor.matmul(lp[:], lkw1[:, mt, :], lkx1[:], start=False, stop=True)
                nc.scalar.activation(
                    lks[:, mt, :], lp[:], AF.Relu, bias=lkb[:, mt : mt + 1]
                )

            w1_sb = hpool.tile([P, 10, 2, P], BF16)
            nc.sync.dma_start(w1_sb[:], w1[:].rearrange("kt mt p q -> p kt mt q"))
            b1_sb = hpool.tile([P, 2], FP32)
            nc.sync.dma_start(b1_sb[:], b1p[:])
            wc_sb = hpool.tile([P, 2], BF16)
            nc.sync.dma_start(wc_sb[:], wc[:])
            bc_sb = hpool.tile([1, 1], FP32)
            nc.sync.dma_start(bc_sb[:], bc_b[:])

            sf, sb_ = fin_hold
            rhs_tiles = []
            for half in range(2):  # code (cols 0:4), comment (cols 4:8)
                c0 = half * BCH
                for dstate in (sf, sb_):
                    for j in range(JH):
                        rhs_tiles.append(dstate[:, j, c0 : c0 + BCH])
            rhs_tiles.append(lks[:, 0, :])
            rhs_tiles.append(lks[:, 1, :])

            yt = hpool.tile([P, 2, BCH], BF16)
            for mt in range(2):
                hp = hppool.tile([P, BCH], FP32, tag="hp")
                for kt in range(10):
                    nc.tensor.matmul(
                        hp[:],
                        w1_sb[:, kt, mt, :],
                        rhs_tiles[kt],
                        start=(kt == 0),
                        stop=(kt == 9),
                    )
                nc.scalar.activation(
                    yt[:, mt, :], hp[:], AF.Relu, bias=b1_sb[:, mt : mt + 1]
                )

            op_ = hppool.tile([1, BCH], FP32, tag="op")
            for kt in range(2):
                nc.tensor.matmul(
                    op_[:],
                    wc_sb[:, kt : kt + 1],
                    yt[:, kt, :],
                    start=(kt == 0),
                    stop=(kt == 1),
                )
            res = hpool.tile([1, BCH], FP32)
            nc.scalar.activation(res[:], op_[:], AF.Sigmoid, bias=bc_sb[0:1, 0:1])
            nc.sync.dma_start(out[:], res[:])

    nc.compile()
    return nc


def prep_inputs(inputs, T=512, n_layers=NLAY):
    """Host-side: shard + pre-layout all tensors. Returns in_maps list."""
    ci = np.asarray(inputs["comment_indices"]).astype(np.int32)
    co = np.asarray(inputs["code_indices"]).astype(np.int32)
    emb_bf = np.ascontiguousarray(
        np.asarray(inputs["embed_table"], np.float32)
    ).astype(NP_BF16)
    gwx = np.asarray(inputs["gru_Wx"], np.float32)
    gwh = np.asarray(inputs["gru_Wh"], np.float32)
    gb = np.asarray(inputs["gru_b"], np.float32)

    wx_t = np.ascontiguousarray(
        gwx[:n_layers].reshape(n_layers, 2, JX, P, JG, P).transpose(0, 1, 2, 4, 3, 5)
    ).astype(NP_BF16)
    wh_t = np.ascontiguousarray(
        gwh[:n_layers].reshape(n_layers, 2, KH, P, JG, P).transpose(0, 1, 2, 4, 3, 5)
    ).astype(NP_BF16)

    pb = gb[:n_layers, :, 0, :].copy()  # [nl, 2, 768]
    pb[:, :, : 2 * H] += gb[:n_layers, :, 1, : 2 * H]
    pbias_h = np.ascontiguousarray(
        pb.reshape(n_layers, 2, JG, P).transpose(3, 0, 1, 2)
    ).astype(np.float32)
    # recurrent h-gate bias broadcast along batch: [P, nl, 2, JH, BC]
    b1h = gb[:n_layers, :, 1, 2 * H :].reshape(n_layers, 2, JH, P)
    bbias_h = np.ascontiguousarray(
        np.broadcast_to(
            b1h.transpose(3, 0, 1, 2)[:, :, :, :, None], (P, n_layers, 2, JH, BC)
        )
    ).astype(NP_BF16)

    s1 = np.asarray(inputs["bn1_gamma"], np.float32) / np.sqrt(
        np.asarray(inputs["bn1_var"], np.float32) + EPS
    )
    t1 = (
        np.asarray(inputs["bn1_beta"], np.float32)
        - np.asarray(inputs["bn1_mean"], np.float32) * s1
    )
    fc1 = np.asarray(inputs["fc1_W"], np.float32)
    w1p = fc1 * s1[:, None]
    b1v = t1 @ fc1 + np.asarray(inputs["fc1_b"], np.float32)
    s2 = np.asarray(inputs["bn2_gamma"], np.float32) / np.sqrt(
        np.asarray(inputs["bn2_var"], np.float32) + EPS
    )
    t2 = (
        np.asarray(inputs["bn2_beta"], np.float32)
        - np.asarray(inputs["bn2_mean"], np.float32) * s2
    )
    clsw = np.asarray(inputs["cls_W"], np.float32)
    wcp = clsw * s2[:, None]
    bcp = (t2 @ clsw + np.asarray(inputs["cls_b"], np.float32)).reshape(1, 1)

    w1_t = np.ascontiguousarray(w1p.reshape(10, P, 2, P).transpose(0, 2, 1, 3)).astype(
        NP_BF16
    )
    b1p_h = np.ascontiguousarray(b1v.reshape(2, P).T).astype(np.float32)
    wc_h = np.ascontiguousarray(wcp.reshape(2, P).T).astype(NP_BF16)

    lw = np.asarray(inputs["leaks_W"], np.float32)
    lw0_h = np.ascontiguousarray(lw[:P].reshape(P, 2, P)).astype(NP_BF16)
    lw1_h = np.ascontiguousarray(lw[P:].reshape(20, 2, P)).astype(NP_BF16)
    lb_h = np.ascontiguousarray(
        np.asarray(inputs["leaks_b"], np.float32).reshape(2, P).T
    ).astype(np.float32)
    leaks = np.asarray(inputs["leaks_indices"], np.float32)

    shared = dict(
        emb=emb_bf, wx=wx_t, wh=wh_t, pbias=pbias_h, bbias=bbias_h,
        w1=w1_t, b1p=b1p_h, wc=wc_h, bc=bcp.astype(np.float32),
        lw0=lw0_h, lw1=lw1_h, lb=lb_h,
    )
    in_maps = []
    for c in range(NCORES):
        exs = slice(BCH * c, BCH * c + BCH)
        merged = np.concatenate([co[exs, :T], ci[exs, :T]], 0)  # [8, T]
        idxw_h = np.ascontiguousarray(
            merged.reshape(BC, T // P, P).transpose(2, 0, 1).reshape(P, -1)
        ).astype(np.int32)
        lkt = np.ascontiguousarray(leaks[exs].T).astype(NP_BF16)
        m = dict(shared)
        m["idxw"] = idxw_h
        m["leakst"] = lkt
        in_maps.append(m)
    return in_maps


def kernel(**inputs) -> np.ndarray:
    from concourse.bass_utils import run_bass_kernel_spmd

    nc = build_nc(T=512)
    in_maps = prep_inputs(inputs, T=512)
    res = run_bass_kernel_spmd(nc, in_maps, list(range(NCORES)))
    outs = [np.asarray(res.results[c]["out"]).reshape(-1) for c in range(NCORES)]
    return np.concatenate(outs).astype(np.float32)


if __name__ == "__main__":
    sys.path.insert(0, "/root/problem")
    import reference

    inp = {k: np.asarray(v) for k, v in reference.setup_inputs().items()}
    got = kernel(**inp)
    print("kernel out:", got[:8])


# revision 23
# speedup vs baseline: 1.5255x; 1.1114x over previous
"""Trainium2 Bass kernel for nn_BaselineModel_27298812133937.

Model: two [32,512] token sequences -> shared embedding [50000,512] ->
3 stacked bi-GRU layers (H=256, Keras reset_after) -> last states,
plus a leaks MLP branch, then BN/FC/BN/sigmoid head -> [32].

Sharding: the two sequences share GRU weights, so they merge into a
batch of 64. Each of the 8 cores takes 8 merged examples (4 code + 4
comment of the same original examples), runs the full network for its
shard with no cross-core communication, and computes the head for its
4 original examples. Host concatenates the 8x[4] outputs.

v3 scan dataflow (trace-driven): tiny matmuls issue every ~27ns on the
PE (the 167ns slice is pipeline latency), so PE work is cheap while the
DVE and the serial gate chain dominate. Per step the precomputed input
projection xz is accumulated into PSUM via an identity matmul, the
recurrent h-gate bias via a K=1 bias-row matmul, so the gate math is
down to sigmoid/tanh on the Act engine plus 5 tensor_tensor ops that
run on DVE for the fw direction and GpSimd for the bw direction. The
new state is written directly into a bf16 staging tile that both the
next step's matmul and the x_next DMA read.
"""

import os
import sys

import numpy as np

for _p in ("/opt/trn_rl_repo",):
    if os.path.isdir(_p) and _p not in sys.path:
        sys.path.insert(0, _p)

import concourse.bass as bass
import concourse.tile as tile
from concourse import bacc, mybir
from concourse.masks import make_identity

import ml_dtypes

FP32 = mybir.dt.float32
BF16 = mybir.dt.bfloat16
I32 = mybir.dt.int32
AF = mybir.ActivationFunctionType
OP = mybir.AluOpType
NP_BF16 = ml_dtypes.bfloat16

V, E, H, NLAY = 50000, 512, 256, 3
EPS = 1e-3
P = 128
JX = E // P        # 4  x-feature tiles
JG = 3 * H // P    # 6  gate tiles
JH = H // P        # 2  hidden tiles
KH = H // P        # 2  Wh contraction tiles
BC = 8             # merged examples per core
BCH = 4            # head (original) examples per core
NCORES = 8
U = 16             # scan unroll == xp time-block


def build_nc(T=512, n_layers=NLAY, use_for_i=True, staggered=True, debug=False):
    assert T % P == 0 and T % (2 * U) == 0
    TB = T // U
    TB2 = T // (2 * U)
    NCH = BC * (T // P)

    nc = bacc.Bacc("TRN2", target_bir_lowering=False, debug=debug)

    def din(name, shape, dt):
        return nc.declare_dram_parameter(name, list(shape), dt, False)

    emb = din("emb", [V, E], BF16)
    idxw = din("idxw", [P, NCH], I32)
    wx = din("wx", [n_layers, 2, JX, JG, P, P], BF16)
    wh = din("wh", [n_layers, 2, KH, JG, P, P], BF16)
    pbias = din("pbias", [P, n_layers, 2, JG], FP32)
    bbias = din("bbias", [P, n_layers, 2, JH, BC], BF16)
    w1 = din("w1", [10, 2, P, P], BF16)
    b1p = din("b1p", [P, 2], FP32)
    wc = din("wc", [P, 2], BF16)
    bc_b = din("bc", [1, 1], FP32)
    lw0 = din("lw0", [P, 2, P], BF16)
    lw1 = din("lw1", [20, 2, P], BF16)
    lb = din("lb", [P, 2], FP32)
    leakst = din("leakst", [148, BCH], BF16)

    out = nc.declare_dram_parameter("out", [1, BCH], FP32, True)

    # internal DRAM
    x_bufs = [nc.dram_tensor(f"x{i}", [P, JX, T, BC], BF16) for i in range(2)]
    # xp: merged projection (zr gates 0:4 with full bias; xh 4:6 with input
    # bias). bw (d=1) stored time-reversed so the scan indexes both dirs
    # identically.
    # xp layout is psum-native [tb, mt, ti, b] so the projection staging
    # write is contiguous; the scan slices per-ti.
    xp = [nc.dram_tensor(f"xp{d}", [P, TB, JG, U, BC], BF16) for d in range(2)]

    with tile.TileContext(nc) as tc, tc.tile_pool(name="const", bufs=1) as cpool:
        # ---- constants in SBUF
        ident = cpool.tile([P, P], BF16)
        make_identity(nc, ident[:])
        idx_sb = cpool.tile([P, NCH], I32)
        nc.sync.dma_start(idx_sb[:], idxw[:])
        pb_sb = cpool.tile([P, n_layers, 2, JG], FP32)
        nc.sync.dma_start(pb_sb[:], pbias[:])
        bb_sb = cpool.tile([P, n_layers, 2, JH, BC], BF16)
        nc.sync.dma_start(bb_sb[:], bbias[:])
        fin_hold = [cpool.tile([P, JH, BC], BF16, name=f"fin{i}") for i in range(2)]

        # ---- phase 1: embedding gather -> x0 (layer-0 input, transposed)
        x0 = x_bufs[0]
        with (
            tc.tile_pool(name="erow", bufs=6) as epool,
            tc.tile_pool(name="estage", bufs=2) as espool,
            tc.tile_pool(name="epsum", bufs=8, space="PSUM") as eppool,
        ):
            for tc_i in range(T // P):
                stages = [
                    espool.tile([P, P, BC], BF16, name=f"estg{j}", tag=f"st{j}")
                    for j in range(JX)
                ]
                for bi in range(BC):
                    ch = bi * (T // P) + tc_i
                    g = epool.tile([P, E], BF16)
                    nc.gpsimd.indirect_dma_start(
                        out=g[:],
                        out_offset=None,
                        in_=emb[:],
                        in_offset=bass.IndirectOffsetOnAxis(
                            ap=idx_sb[:, ch : ch + 1], axis=0
                        ),
                    )
                    for j in range(JX):
                        pst = eppool.tile([P, P], BF16)
                        nc.tensor.transpose(pst[:], g[:, j * P : (j + 1) * P], ident[:])
                        if (bi + j) % 2 == 0:
                            nc.vector.tensor_copy(stages[j][:, :, bi], pst[:])
                        else:
                            nc.scalar.copy(stages[j][:, :, bi], pst[:])
                for j in range(JX):
                    nc.sync.dma_start(
                        x0[:, j, tc_i * P : (tc_i + 1) * P, :], stages[j][:]
                    )

        # ---- per-layer: proj (both dirs) then scan (both dirs)
        for l in range(n_layers):
            x_cur = x_bufs[l % 2]
            x_next = x_bufs[(l + 1) % 2]
            is_last = l == n_layers - 1

            # -- input projection: xp^T = Wx^T @ x^T (+bias), to DRAM (bf16)
            with (
                tc.tile_pool(name="wts", bufs=1) as wpool,
                tc.tile_pool(name="pstage", bufs=3) as pspool,
                tc.tile_pool(name="ppsum", bufs=2, space="PSUM") as pppool,
                tc.tile_pool(name="xchunk", bufs=2) as xcpool,
            ):
                wx_sb = wpool.tile([P, 2, JX, JG, P], BF16)
                nc.sync.dma_start(wx_sb[:], wx[l].rearrange("d kt mt p q -> p d kt mt q"))

                NCK = T // 64  # chunks of 512 cols (64 t x 8 b)
                for d in range(2):
                    for c in range(NCK):
                        xch = xcpool.tile([P, JX, 64, BC], BF16, tag="xch")
                        nc.sync.dma_start(xch[:], x_cur[:, :, c * 64 : (c + 1) * 64, :])
                        for mt in range(JG):
                            ps = pppool.tile([P, 512], FP32)
                            for kt in range(JX):
                                nc.tensor.matmul(
                                    ps[:],
                                    wx_sb[:, d, kt, mt, :],
                                    xch[:, kt, :, :],
                                    start=(kt == 0),
                                    stop=(kt == JX - 1),
                                )
                            # staging layout matches psum iteration order
                            # (tb, ti, b) so the write is contiguous
                            NTB = 64 // U
                            stg = pspool.tile([P, NTB, U, BC], BF16, tag="stg")
                            if d == 0:
                                stg_w = stg[:]
                                tb_lo = c * NTB
                                dst_tb = slice(tb_lo, tb_lo + NTB)
                            else:
                                # bw: store reversed in time (block and
                                # within-block order both reversed) by writing
                                # the staging tile through a reversed AP
                                stg_w = stg[:, ::-1, ::-1, :]
                                tb_hi = TB - c * NTB
                                dst_tb = slice(tb_hi - NTB, tb_hi)
                            if mt % 2 == 0:
                                nc.vector.tensor_scalar_add(
                                    stg_w, ps[:], pb_sb[:, l, d, mt : mt + 1]
                                )
                            else:
                                nc.scalar.activation(
                                    stg_w,
                                    ps[:],
                                    AF.Identity,
                                    bias=pb_sb[:, l, d, mt : mt + 1],
                                )
                            # xp slot order: r gates (mt 2,3) first, then z
                            # (mt 0,1), then h (mt 4,5) — the scan reads
                            # contiguous [0:4] for the rz psum and [4:6] for xh.
                            pos = {2: 0, 3: 1, 0: 2, 1: 3, 4: 4, 5: 5}[mt]
                            nc.sync.dma_start(xp[d][:, dst_tb, pos, :, :], stg[:])

            # -- scan
            with (
                tc.tile_pool(name="state", bufs=4) as stpool,
                tc.tile_pool(name="gates", bufs=4) as gpool,
                tc.tile_pool(name="xpchunk", bufs=3) as xppool,
                tc.tile_pool(name="spsum", bufs=2, space="PSUM") as sppool,
                tc.tile_pool(name="wts2", bufs=1) as wpool2,
            ):
                wh_sb = wpool2.tile([P, 2, KH, JG, P], BF16)
                nc.sync.dma_start(wh_sb[:], wh[l].rearrange("d kt mt p q -> p d kt mt q"))

                # fixed staging tiles: state slots double as x_next staging.
                # body handles 2 sub-blocks (A then B); A's u=0 reads B's
                # last slot from the previous iteration.
                stg = []  # [d][sub] -> tile [P, JH, U, BC] bf16
                for d in range(2):
                    sA = stpool.tile([P, JH, U, BC], BF16, name=f"stgA{d}")
                    sB = stpool.tile([P, JH, U, BC], BF16, name=f"stgB{d}")
                    nc.vector.memset(sB[:], 0.0)
                    stg.append((sA, sB))

                def scan_sub(ib, sub, dyn):
                    # chunk loads (one per dir): [P, JG, BC, U] bf16
                    chunks = []
                    for d in range(2):
                        if dyn:
                            tbs = bass.ds(ib * 2 + sub, 1)
                        else:
                            tbs = slice(ib * 2 + sub, ib * 2 + sub + 1)
                        xc = xppool.tile([P, 1, JG, U, BC], BF16, tag=f"xc{d}{sub}")
                        nc.sync.dma_start(xc[:], xp[d][:, tbs, :, :, :])
                        chunks.append(xc)
                    cur = [stg[d][sub] for d in range(2)]
                    prev = [stg[d][1 - sub] for d in range(2)]
                    for u in range(U):
                        vprevs, slots = [], []
                        for d in range(2):
                            # fw fills slots ascending (slot u == time base+u);
                            # bw fills descending (slot U-1-u) so the tile ends
                            # up in ascending-time order for a direct DMA.
                            if d == 0:
                                slots.append(u)
                                vprevs.append(
                                    cur[d][:, :, u - 1, :]
                                    if u > 0
                                    else prev[d][:, :, U - 1, :]
                                )
                            else:
                                slots.append(U - 1 - u)
                                vprevs.append(
                                    cur[d][:, :, U - u, :]
                                    if u > 0
                                    else prev[d][:, :, 0, :]
                                )
                        # rz psum (one wide identity + 8 wh mms, r-tiles first),
                        # then the h psum (identity-bias + 4 wh); all
                        # stationaries are K=128 so the PE never switches tile
                        # geometry.
                        ptrz_, pth_ = [], []
                        for d in range(2):
                            # [0:2] = r gates (mt 2,3), [2:4] = z gates (mt 0,1)
                            xz_rz = chunks[d][:, 0, 0:4, u, :]
                            prz = sppool.tile([P, 4, BC], FP32, tag=f"prz{d}")
                            nc.tensor.matmul(
                                prz[:], ident[:], xz_rz, start=True, stop=False,
                                skip_group_check=True,
                            )
                            for i, mt in enumerate((2, 3, 0, 1)):
                                for kt in range(KH):
                                    nc.tensor.matmul(
                                        prz[:, i, :],
                                        wh_sb[:, d, kt, mt, :],
                                        vprevs[d][:, kt, :],
                                        start=False, stop=(kt == KH - 1),
                                        skip_group_check=True,
                                    )
                            ptrz_.append(prz)
                        for d in range(2):
                            pth = sppool.tile([P, 2, BC], FP32, tag=f"ph{d}")
                            nc.tensor.matmul(
                                pth[:], ident[:], bb_sb[:, l, d, :, :],
                                start=True, stop=False, skip_group_check=True,
                            )
                            for jh in range(JH):
                                for kt in range(KH):
                                    nc.tensor.matmul(
                                        pth[:, jh, :],
                                        wh_sb[:, d, kt, 4 + jh, :],
                                        vprevs[d][:, kt, :],
                                        start=False, stop=(kt == KH - 1),
                                        skip_group_check=True,
                                    )
                            pth_.append(pth)
                        for d in range(2):
                            vprev, slot_w = vprevs[d], slots[d]
                            xh = chunks[d][:, 0, 4:6, u, :]
                            # gpsimd cannot touch PSUM and is slow on
                            # tensor_scalar forms: hm and g always on DVE, the
                            # rest of the bw chain on gpsimd.
                            eng = nc.vector if d == 0 else nc.gpsimd
                            rz = gpool.tile([P, 4, BC], FP32, tag=f"rz{d}")
                            nc.scalar.activation(rz[:], ptrz_[d][:], AF.Sigmoid)
                            # off-chain helper: w = z*vprev
                            w_ = gpool.tile([P, 2, BC], FP32, tag=f"w{d}")
                            eng.tensor_tensor(w_[:], rz[:, 2:4, :], vprev, OP.mult)
                            # critical tail
                            hm = gpool.tile([P, 2, BC], FP32, tag=f"hm{d}")
                            nc.vector.tensor_tensor(
                                hm[:], pth_[d][:], rz[:, 0:2, :], OP.mult
                            )
                            av = gpool.tile([P, 2, BC], FP32, tag=f"av{d}")
                            eng.tensor_tensor(av[:], hm[:], xh, OP.add)
                            hh = gpool.tile([P, 2, BC], FP32, tag=f"hh{d}")
                            nc.scalar.activation(hh[:], av[:], AF.Tanh)
                            # g = (z-1)*hh in one stt on DVE
                            g_ = gpool.tile([P, 2, BC], FP32, tag=f"g{d}")
                            nc.vector.scalar_tensor_tensor(
                                g_[:], rz[:, 2:4, :], -1.0, hh[:], OP.add, OP.mult
                            )
                            # v = w - g = z*vprev + (1-z)*hh
                            eng.tensor_tensor(
                                cur[d][:, :, slot_w, :], w_[:], g_[:], OP.subtract
                            )
                    if not is_last:
                        for d in range(2):
                            if d == 0:
                                if dyn:
                                    t_ap = bass.ds(ib * (2 * U) + sub * U, U)
                                else:
                                    t0 = ib * 2 * U + sub * U
                                    t_ap = slice(t0, t0 + U)
                            else:
                                if dyn:
                                    t_ap = bass.ds(
                                        ib * (-2 * U) + (T - U - sub * U), U
                                    )
                                else:
                                    t0 = T - U - sub * U - ib * 2 * U
                                    t_ap = slice(t0, t0 + U)
                            nc.sync.dma_start(
                                x_next[:, 2 * d : 2 * d + 2, t_ap, :], cur[d][:]
                            )

                if use_for_i:
                    with tc.For_i(0, TB2, 1, staggered_reset=staggered) as ib:
                        scan_sub(ib, 0, True)
                        scan_sub(ib, 1, True)
                else:
                    for ib in range(TB2):
                        scan_sub(ib, 0, False)
                        scan_sub(ib, 1, False)

                if is_last:
                    # final state: fw in last slot, bw in slot 0 (descending fill)
                    nc.vector.tensor_copy(fin_hold[0][:], stg[0][1][:, :, U - 1, :])
                    nc.vector.tensor_copy(fin_hold[1][:], stg[1][1][:, :, 0, :])

        # ---- head: leaks branch + folded BN/FC/BN/sigmoid
        with (
            tc.tile_pool(name="head", bufs=1) as hpool,
            tc.tile_pool(name="hpsum", bufs=2, space="PSUM") as hppool,
        ):
            lkw0 = hpool.tile([P, 2, P], BF16)
            nc.sync.dma_start(lkw0[:], lw0[:])
            lkw1 = hpool.tile([20, 2, P], BF16)
            nc.sync.dma_start(lkw1[:], lw1[:])
            lkb = hpool.tile([P, 2], FP32)
            nc.sync.dma_start(lkb[:], lb[:])
            lkx0 = hpool.tile([P, BCH], BF16)
            nc.sync.dma_start(lkx0[:], leakst[0:P, :])
            lkx1 = hpool.tile([20, BCH], BF16)
            nc.sync.dma_start(lkx1[:], leakst[P:148, :])

            lks = hpool.tile([P, 2, BCH], BF16)
            for mt in range(2):
                lp = hppool.tile([P, BCH], FP32, tag="lp")
                nc.tensor.matmul(lp[:], lkw0[:, mt, :], lkx0[:], start=True, stop=False)
                nc.tensor.matmul(lp[:], lkw1[:, mt, :], lkx1[:], start=False, stop=True)
                nc.scalar.activation(
                    lks[:, mt, :], lp[:], AF.Relu, bias=lkb[:, mt : mt + 1]
                )

            w1_sb = hpool.tile([P, 10, 2, P], BF16)
            nc.sync.dma_start(w1_sb[:], w1[:].rearrange("kt mt p q -> p kt mt q"))
            b1_sb = hpool.tile([P, 2], FP32)
            nc.sync.dma_start(b1_sb[:], b1p[:])
            wc_sb = hpool.tile([P, 2], BF16)
            nc.sync.dma_start(wc_sb[:], wc[:])
            bc_sb = hpool.tile([1, 1], FP32)
            nc.sync.dma_start(bc_sb[:], bc_b[:])

            sf, sb_ = fin_hold
            rhs_tiles = []
            for half in range(2):  # code (cols 0:4), comment (cols 4:8)
                c0 = half * BCH
                for dstate in (sf, sb_):
                    for j in range(JH):
                        rhs_tiles.append(dstate[:, j, c0 : c0 + BCH])
            rhs_tiles.append(lks[:, 0, :])
            rhs_tiles.append(lks[:, 1, :])

            yt = hpool.tile([P, 2, BCH], BF16)
            for mt in range(2):
                hp = hppool.tile([P, BCH], FP32, tag="hp")
                for kt in range(10):
                    nc.tensor.matmul(
                        hp[:],
                        w1_sb[:, kt, mt, :],
                        rhs_tiles[kt],
                        start=(kt == 0),
                        stop=(kt == 9),
                    )
                nc.scalar.activation(
                    yt[:, mt, :], hp[:], AF.Relu, bias=b1_sb[:, mt : mt + 1]
                )

            op_ = hppool.tile([1, BCH], FP32, tag="op")
            for kt in range(2):
                nc.tensor.matmul(
                    op_[:],
                    wc_sb[:, kt : kt + 1],
                    yt[:, kt, :],
                    start=(kt == 0),
                    stop=(kt == 1),
                )
            res = hpool.tile([1, BCH], FP32)
            nc.scalar.activation(res[:], op_[:], AF.Sigmoid, bias=bc_sb[0:1, 0:1])
            nc.sync.dma_start(out[:], res[:])

    nc.compile()
    return nc


def prep_inputs(inputs, T=512, n_layers=NLAY):
    """Host-side: shard + pre-layout all tensors. Returns in_maps list."""
    ci = np.asarray(inputs["comment_indices"]).astype(np.int32)
    co = np.asarray(inputs["code_indices"]).astype(np.int32)
    emb_bf = np.ascontiguousarray(
        np.asarray(inputs["embed_table"], np.float32)
    ).astype(NP_BF16)
    gwx = np.asarray(inputs["gru_Wx"], np.float32)
    gwh = np.asarray(inputs["gru_Wh"], np.float32)
    gb = np.asarray(inputs["gru_b"], np.float32)

    wx_t = np.ascontiguousarray(
        gwx[:n_layers].reshape(n_layers, 2, JX, P, JG, P).transpose(0, 1, 2, 4, 3, 5)
    ).astype(NP_BF16)
    wh_t = np.ascontiguousarray(
        gwh[:n_layers].reshape(n_layers, 2, KH, P, JG, P).transpose(0, 1, 2, 4, 3, 5)
    ).astype(NP_BF16)

    pb = gb[:n_layers, :, 0, :].copy()  # [nl, 2, 768]
    pb[:, :, : 2 * H] += gb[:n_layers, :, 1, : 2 * H]
    pbias_h = np.ascontiguousarray(
        pb.reshape(n_layers, 2, JG, P).transpose(3, 0, 1, 2)
    ).astype(np.float32)
    # recurrent h-gate bias broadcast along batch: [P, nl, 2, JH, BC]
    b1h = gb[:n_layers, :, 1, 2 * H :].reshape(n_layers, 2, JH, P)
    bbias_h = np.ascontiguousarray(
        np.broadcast_to(
            b1h.transpose(3, 0, 1, 2)[:, :, :, :, None], (P, n_layers, 2, JH, BC)
        )
    ).astype(NP_BF16)

    s1 = np.asarray(inputs["bn1_gamma"], np.float32) / np.sqrt(
        np.asarray(inputs["bn1_var"], np.float32) + EPS
    )
    t1 = (
        np.asarray(inputs["bn1_beta"], np.float32)
        - np.asarray(inputs["bn1_mean"], np.float32) * s1
    )
    fc1 = np.asarray(inputs["fc1_W"], np.float32)
    w1p = fc1 * s1[:, None]
    b1v = t1 @ fc1 + np.asarray(inputs["fc1_b"], np.float32)
    s2 = np.asarray(inputs["bn2_gamma"], np.float32) / np.sqrt(
        np.asarray(inputs["bn2_var"], np.float32) + EPS
    )
    t2 = (
        np.asarray(inputs["bn2_beta"], np.float32)
        - np.asarray(inputs["bn2_mean"], np.float32) * s2
    )
    clsw = np.asarray(inputs["cls_W"], np.float32)
    wcp = clsw * s2[:, None]
    bcp = (t2 @ clsw + np.asarray(inputs["cls_b"], np.float32)).reshape(1, 1)

    w1_t = np.ascontiguousarray(w1p.reshape(10, P, 2, P).transpose(0, 2, 1, 3)).astype(
        NP_BF16
    )
    b1p_h = np.ascontiguousarray(b1v.reshape(2, P).T).astype(np.float32)
    wc_h = np.ascontiguousarray(wcp.reshape(2, P).T).astype(NP_BF16)

    lw = np.asarray(inputs["leaks_W"], np.float32)
    lw0_h = np.ascontiguousarray(lw[:P].reshape(P, 2, P)).astype(NP_BF16)
    lw1_h = np.ascontiguousarray(lw[P:].reshape(20, 2, P)).astype(NP_BF16)
    lb_h = np.ascontiguousarray(
        np.asarray(inputs["leaks_b"], np.float32).reshape(2, P).T
    ).astype(np.float32)
    leaks = np.asarray(inputs["leaks_indices"], np.float32)

    shared = dict(
        emb=emb_bf, wx=wx_t, wh=wh_t, pbias=pbias_h, bbias=bbias_h,
        w1=w1_t, b1p=b1p_h, wc=wc_h, bc=bcp.astype(np.float32),
        lw0=lw0_h, lw1=lw1_h, lb=lb_h,
    )
    in_maps = []
    for c in range(NCORES):
        exs = slice(BCH * c, BCH * c + BCH)
        merged = np.concatenate([co[exs, :T], ci[exs, :T]], 0)  # [8, T]
        idxw_h = np.ascontiguousarray(
            merged.reshape(BC, T // P, P).transpose(2, 0, 1).reshape(P, -1)
        ).astype(np.int32)
        lkt = np.ascontiguousarray(leaks[exs].T).astype(NP_BF16)
        m = dict(shared)
        m["idxw"] = idxw_h
        m["leakst"] = lkt
        in_maps.append(m)
    return in_maps


def kernel(**inputs) -> np.ndarray:
    from concourse.bass_utils import run_bass_kernel_spmd

    nc = build_nc(T=512)
    in_maps = prep_inputs(inputs, T=512)
    res = run_bass_kernel_spmd(nc, in_maps, list(range(NCORES)))
    outs = [np.asarray(res.results[c]["out"]).reshape(-1) for c in range(NCORES)]
    return np.concatenate(outs).astype(np.float32)


if __name__ == "__main__":
    sys.path.insert(0, "/root/problem")
    import reference

    inp = {k: np.asarray(v) for k, v in reference.setup_inputs().items()}
    got = kernel(**inp)
    print("kernel out:", got[:8])
